# revision 35
# baseline (speedup 1.0000x reference)
"""Trainium2 Bass kernel for nn_CSTR: B=4096-lane vmapped 2047-step rollout.

v3: data-parallel over 8 cores (512 lanes each), 2 independent 256-lane
chains per core, emission-staggered half a step apart so their
instruction streams fill each other's dependency gaps.

Per chain: slot-major layout, slot = 8 partitions x 32 free lanes,
state slots (x1, x2, xh1, xh2). Mega input tile X[104,32] bf16 =
[state@0; tanh@32; tanh^2@64; ones@96] (every engine-written sub-view
32-partition aligned). The full state update collapses to ONE bf16
matmul producing the O(H) increment Delta (u = K@xhat composed in, RK4
constants on the ones slot); the exact fp32 state G[32,32] is kept via
a vector add G' = Delta + (G + w). Gating: W_VD -> [Ls@rx + M;
0pad+(x-fp); Qc@x], products on DVE, W_PH -> phi (x4 dup), sigmoid on
ScalarE; the blend add uses a zero-padded delta*(x-fp) [32,32] so both
the fp32 state and bf16 mirror update with one aligned add each.
Stage cost accumulates into a persistent PSUM bank every 4 steps from
two [128,32] staging tiles (deltas, Qc products). Only O(H)-scaled
terms flow through bf16 matmuls; fp32 trajectory exactness is
preserved via the vector adds (2.7e-4 max rel err vs reference).
"""
import os
import sys
import numpy as np
import ml_dtypes
from contextlib import ExitStack

sys.path.insert(0, "/opt/trn_rl_repo")

import concourse.bacc as bacc
import concourse.bass as bass
import concourse.mybir as mybir
import concourse.tile as tile
from concourse import bass_utils

F32 = mybir.dt.float32
BF16 = mybir.dt.bfloat16
AF = mybir.ActivationFunctionType

H = np.float32(0.01)
LAM = np.float32(1.0)
B_TOT, N_CORES = 4096, 8
LPC = 512                 # lanes per core
NCH = 2                   # chains per core
LCH = 256                 # lanes per chain
NG, NL = 8, 32            # groups x free lanes per chain
TR = 2047                 # real steps
CH = 256                  # steps per w chunk
NQ = 8                    # chunks (last holds 255 real steps)

C1 = np.float32(1.0) - H
GC = np.float32(0.5) * H
EC = np.float32(H * H / 2)
A0 = np.float32(5e-5)
ALPHA = np.float32(H - np.float32(1e-6) / 3)


def _kron8(A):
    """lhsT for slot map A[out_slot, in_slot] -> [8*in, 8*out] bf16."""
    k = np.kron(np.ascontiguousarray(A.T), np.eye(NG, dtype=np.float32))
    return k.astype(ml_dtypes.bfloat16)


def _build_weights(K, L, M, Mo):
    K1, K2 = np.float32(K[0, 0]), np.float32(K[0, 1])
    Ls = ((L + L.T) * np.float32(0.5)).astype(np.float32)
    Mv = M[0].astype(np.float32)
    Qc = np.array([[1 + K1 * K1, K1 * K2], [K1 * K2, 1 + K2 * K2]], np.float32)
    # slot order (x1, x2, xh1, xh2) == rx order (x1, x2, fp1, fp2)

    # W_X inputs: state(4) T(4) Q(4) ones(1) = 13 slots -> Delta(4)
    uc = np.array([H, GC, H, GC], np.float32)
    Kv = np.array([0, 0, K1, K2], np.float32)
    A_S = (C1 - 1) * np.eye(4, dtype=np.float32) + np.outer(uc, Kv)
    A_T = np.zeros((4, 4), np.float32)
    A_T[0, 1] = ALPHA; A_T[1, 0] = -H; A_T[2, 3] = ALPHA; A_T[3, 2] = -H
    A_Q = np.zeros((4, 4), np.float32)
    A_Q[0, 1] = -A0; A_Q[2, 3] = -A0
    A_ONES = np.array([A0, -EC, A0, -EC], np.float32).reshape(4, 1)
    WX = np.concatenate([A_S, A_T, A_Q, A_ONES], axis=1)   # [4, 13]

    # W_VD: 13 input slots -> [y(4); 0pad(2); DIF(2); y2(2)] = 10 out,
    # composed with the state update so it reads the PRE-update X tile:
    # VD = (Avd + AV4@A_WX)@X_old + AV4@w  (exact linear algebra).
    Avd = np.zeros((10, 13), np.float32)
    for s in range(4):
        for sp in range(4):
            Avd[s, sp] = Ls[s, sp]
        Avd[s, 12] = Mv[s]
    Avd[6, 0] = 1; Avd[6, 2] = -1          # DIF1 = x1 - xh1
    Avd[7, 1] = 1; Avd[7, 3] = -1          # DIF2 = x2 - xh2
    Avd[8, 0] = Qc[0, 0]; Avd[8, 1] = Qc[0, 1]
    Avd[9, 0] = Qc[1, 0]; Avd[9, 1] = Qc[1, 1]
    AV4 = Avd[:, 0:4]
    WVDX = Avd + AV4 @ WX                  # [10, 13]
    WVDW = AV4                             # [10, 4] applied to (w1,w2,0,0)
    # single fused matmul: out slots [VD(10); pad(2); Delta(4)] = 16
    WBIG = np.concatenate(
        [WVDX, np.zeros((2, 13), np.float32), WX], axis=0)   # [16, 13]

    APH = np.ones((4, 4), np.float32)      # phi, x4 dup
    AC4D = np.zeros((1, 16), np.float32)
    AC4P = np.zeros((1, 16), np.float32)
    for b in range(4):
        AC4D[0, 4 * b] = LAM
        AC4P[0, 4 * b] = 1; AC4P[0, 4 * b + 1] = 1
    AC1D = np.zeros((1, 4), np.float32); AC1D[0, 0] = LAM
    AC1P = np.zeros((1, 4), np.float32); AC1P[0, 0] = 1; AC1P[0, 1] = 1
    AFIN = np.array([[10.0, 10.0]], np.float32)

    W = {
        "WX": _kron8(WX),        # [104, 32]
        "WVD": _kron8(WVDX),     # [104, 80]
        "WVDW": _kron8(WVDW),    # [32, 80]
        "WPH": _kron8(APH),      # [32, 32]
        "WAC4D": _kron8(AC4D),   # [128, 8]
        "WAC4P": _kron8(AC4P),   # [128, 8]
        "WAC1D": _kron8(AC1D),   # [32, 8]
        "WAC1P": _kron8(AC1P),   # [32, 8]
        "WFIN": _kron8(AFIN),    # [16, 8]
    }
    init_c = float(1.0 + K1 * K1 + LAM)
    return W, float(Mo[0, 0]), init_c


WSHAPES = {"WX": (104, 32), "WVD": (104, 80), "WVDW": (32, 80),
           "WPH": (32, 32),
           "WAC4D": (128, 8), "WAC4P": (128, 8), "WAC1D": (32, 8),
           "WAC1P": (32, 8), "WFIN": (16, 8)}


def _init_consts():
    xa = np.zeros((104, NL), np.float32)
    for lo, v in ((0, 1.0), (8, 0.0), (16, 1.0), (24, 0.0)):   # state
        xa[lo:lo + 8] = v
    xa[96:104] = 1.0                                           # ones
    xb = np.zeros((104, NL), np.float32)
    xb[96:104] = 1.0
    gi = np.zeros((32, NL), np.float32)
    gi[0:8] = 1.0; gi[16:24] = 1.0
    return (xa.astype(ml_dtypes.bfloat16), xb.astype(ml_dtypes.bfloat16), gi)


def _build_program(Mo_f, init_c):
    nc = bacc.Bacc("TRN2", target_bir_lowering=False, debug=False)

    wd = {}
    for c in range(NCH):
        for q in range(NQ):
            wd[(c, q)] = nc.dram_tensor(
                f"w{c}_{q}", [32, CH * NL], BF16, kind="ExternalInput")
    wdram = {n: nc.dram_tensor(n, list(WSHAPES[n]), BF16, kind="ExternalInput")
             for n in WSHAPES}
    xa_d = nc.dram_tensor("XIA", [104, NL], BF16, kind="ExternalInput")
    xb_d = nc.dram_tensor("XIB", [104, NL], BF16, kind="ExternalInput")
    gi_d = nc.dram_tensor("GI", [32, NL], F32, kind="ExternalInput")
    out_d = [nc.dram_tensor(f"out{c}", [NG, NL], F32, kind="ExternalOutput")
             for c in range(NCH)]

    ctx = ExitStack()
    with tile.TileContext(nc) as tc:
        with tc.tile_pool(name="wq", bufs=2) as wpool, \
             tc.tile_pool(name="wt", bufs=1) as cpool, \
             tc.tile_pool(name="sb", bufs=3) as sp, \
             tc.tile_pool(name="ps", bufs=1, space="PSUM") as pp, \
             tc.tile_pool(name="pacc", bufs=1, space="PSUM") as pa:

            wt = {}
            for n in WSHAPES:
                wt[n] = cpool.tile(list(WSHAPES[n]), BF16, tag=n, name=n + "_t")
                nc.sync.dma_start(wt[n][:, :], wdram[n].ap())

            wtiles = {}

            def get_wtile(c, q):
                if (c, q) not in wtiles:
                    t = wpool.tile([32, CH * NL], BF16, tag=f"w{c}",
                                   name=f"w{c}_{q}_t")
                    nc.sync.dma_start(t[:, :], wd[(c, q)].ap())
                    wtiles[(c, q)] = t
                return wtiles[(c, q)]

            chs = []
            for c in range(NCH):
                S = {}
                S["X"] = [cpool.tile([104, NL], BF16, tag=f"XA{c}", name=f"XA{c}"),
                          cpool.tile([104, NL], BF16, tag=f"XB{c}", name=f"XB{c}")]
                S["G"] = [cpool.tile([32, NL], F32, tag=f"GA{c}", name=f"GA{c}"),
                          cpool.tile([32, NL], F32, tag=f"GB{c}", name=f"GB{c}")]
                S["RW"] = cpool.tile([32, NL], F32, tag=f"RW{c}", name=f"RW{c}")
                S["SG"] = cpool.tile([128, NL], BF16, tag=f"SG{c}", name=f"SG{c}")
                S["SP"] = cpool.tile([128, NL], BF16, tag=f"SP{c}", name=f"SP{c}")
                S["MOB"] = cpool.tile([32, 1], F32, tag=f"MOB{c}", name=f"MOB{c}")
                S["ACC"] = pa.tile([8, NL], F32, tag=f"ACC{c}", name=f"ACC{c}")
                S["tagD"] = f"Dp{c}"
                S["tagV"] = f"VDp{c}"
                S["tagP"] = f"PHp{c}"
                S["tagPP"] = f"PP{c}"
                S["tagM"] = f"M1{c}"
                chs.append(S)

            Z = cpool.tile([32, NL], BF16, tag="Z", name="Z")
            nc.vector.memset(Z[:, :], 0.0)

            for c, S in enumerate(chs):
                nc.sync.dma_start(S["X"][0][:, :], xa_d.ap())
                nc.sync.dma_start(S["X"][1][:, :], xb_d.ap())
                nc.sync.dma_start(S["G"][0][:, :], gi_d.ap())
                nc.vector.memset(S["SG"][:, :], 0.0)
                nc.gpsimd.memset(S["SP"][:, :], 0.0)
                nc.vector.memset(S["MOB"][:, :], Mo_f)
                nc.tensor.matmul(S["ACC"][:, :], wt["WAC1D"][:, :], Z[:, :],
                                 start=True, stop=False)
                w0 = get_wtile(c, 0)
                nc.vector.tensor_add(S["RW"][:, :], S["G"][0][:, :],
                                     w0[:, 0:NL])
                if c == 1:
                    # one-time ~1.8us delay on chain B's first state add:
                    # forces the two chains into anti-phase so their ops
                    # fill each other's dependency gaps instead of
                    # contending in lockstep.
                    for _ in range(10):
                        nc.vector.tensor_add(S["RW"][:, :], S["RW"][:, :],
                                             Z[:, :])
                get_wtile(c, 1)

            def step_ops(S, p, wv_cur, wv_next, b, do_gate, do_cost, do_acc):
                """One step of one chain; p = input parity, b = SG block."""
                Xi, Xo = S["X"][p], S["X"][1 - p]
                Go = S["G"][1 - p]
                RW, SG, SP = S["RW"], S["SG"], S["SP"]
                Dp = pp.tile([32, NL], F32, tag=S["tagD"])
                ops = []
                ops.append(lambda: nc.scalar.activation(
                    Xi[32:64, :], Xi[0:32, :], AF.Tanh))
                ops.append(lambda: nc.scalar.activation(
                    Xi[64:96, :], Xi[32:64, :], AF.Square))
                ops.append(lambda: nc.tensor.matmul(
                    Dp[:, :], wt["WX"][:, :], Xi[:, :],
                    start=True, stop=True))
                if do_gate:
                    VDp = pp.tile([80, NL], F32, tag=S["tagV"])
                    ops.append(lambda: nc.tensor.matmul(
                        VDp[:, :], wt["WVD"][:, :], Xi[:, :],
                        start=True, stop=False))
                    ops.append(lambda: nc.tensor.matmul(
                        VDp[:, :], wt["WVDW"][:, :], wv_cur,
                        start=False, stop=True))
                ops.append(lambda: nc.vector.tensor_add(
                    Xo[0:32, :], Dp[:, :], RW[:, :]))
                ops.append(lambda: nc.vector.tensor_add(
                    Go[:, :], Dp[:, :], RW[:, :]))
                if do_gate:
                    PHp = pp.tile([32, NL], F32, tag=S["tagP"])
                    PP = sp.tile([32, NL], BF16, tag=S["tagPP"])
                    M1 = sp.tile([32, NL], F32, tag=S["tagM"])
                    ops.append(lambda: nc.vector.tensor_mul(
                        PP[:, :], Xo[0:32, :], VDp[0:32, :]))
                    if do_cost:
                        ops.append(lambda: nc.vector.tensor_mul(
                            SP[32 * b:32 * b + 16, :],
                            Xo[0:16, :], VDp[64:80, :]))
                    ops.append(lambda: nc.tensor.matmul(
                        PHp[:, :], wt["WPH"][:, :], PP[:, :],
                        start=True, stop=True))
                    ops.append(lambda: nc.scalar.activation(
                        SG[32 * b:32 * b + 32, :], PHp[:, :], AF.Sigmoid,
                        bias=S["MOB"][:, :]))
                    ops.append(lambda: nc.vector.tensor_mul(
                        M1[:, :], SG[32 * b:32 * b + 32, :], VDp[32:64, :]))
                    ops.append(lambda: nc.vector.tensor_add(
                        Xo[0:32, :], Xo[0:32, :], M1[:, :]))
                    ops.append(lambda: nc.gpsimd.tensor_add(
                        Go[:, :], Go[:, :], M1[:, :]))
                    if do_acc:
                        ops.append(lambda: nc.tensor.matmul(
                            S["ACC"][:, :], wt["WAC4D"][:, :], SG[:, :],
                            start=False, stop=False))
                        ops.append(lambda: nc.tensor.matmul(
                            S["ACC"][:, :], wt["WAC4P"][:, :], SP[:, :],
                            start=False, stop=False))
                if wv_next is not None:
                    ops.append(lambda: nc.gpsimd.tensor_add(
                        RW[:, :], Go[:, :], wv_next))
                return ops

            def emit_group(k0, wts, wts_next_chunk):
                """4 steps (k0..k0+3) for both chains, B staggered."""
                for j in range(4):
                    k = k0 + j
                    do_gate = k <= TR - 2
                    do_cost = k <= TR - 3
                    do_acc = do_cost and (j == 3)
                    opsl = []
                    for c, S in enumerate(chs):
                        wtile, base = wts[c]
                        if isinstance(base, int):
                            wv_cur = wtile[:, base + j * NL:base + (j + 1) * NL]
                        else:
                            wv_cur = wtile[:, bass.ds(base + j * NL, NL)]
                        if k + 1 <= TR - 1:
                            if j == 3 and wts_next_chunk is not None:
                                nwtile, nbase = wts_next_chunk[c]
                                wv_next = nwtile[:, nbase:nbase + NL]
                            elif isinstance(base, int):
                                nb = base + (j + 1) * NL
                                wv_next = wtile[:, nb:nb + NL]
                            else:
                                wv_next = wtile[:, bass.ds(base + (j + 1) * NL, NL)]
                        else:
                            wv_next = None
                        opsl.append(step_ops(S, j % 2, wv_cur, wv_next, j,
                                             do_gate, do_cost, do_acc))
                    sa, sb = opsl
                    off = 8   # stagger chain B ~half a step behind A
                    for i in range(max(len(sa), len(sb)) + off):
                        if i < len(sa):
                            sa[i]()
                        if 0 <= i - off < len(sb):
                            sb[i - off]()

            # chunks of 256 steps: 63 hw-loop groups + 4 static tail steps
            for q in range(NQ):
                for c in range(NCH):
                    get_wtile(c, q)
                with tc.For_i(0, 63, 1) as iv:
                    wts = [(wtiles[(c, q)], iv * (4 * NL)) for c in range(NCH)]
                    emit_group(q * CH, wts, None)
                k0 = q * CH + 252
                nxt = None
                if q + 1 < NQ:
                    nxt = [(get_wtile(c, q + 1), 0) for c in range(NCH)]
                wts = [(wtiles[(c, q)], 252 * NL) for c in range(NCH)]
                emit_group(k0, wts, nxt)

            # epilogue: after 2047 steps state parity lands in G[1]
            for c, S in enumerate(chs):
                Gl = S["G"][1]
                FSQ = sp.tile([16, NL], BF16, tag=f"FSQ{c}")
                nc.vector.tensor_mul(FSQ[:, :], Gl[0:16, :], Gl[0:16, :])
                nc.tensor.matmul(S["ACC"][:, :], wt["WFIN"][:, :], FSQ[:, :],
                                 start=False, stop=False)
                nc.tensor.matmul(S["ACC"][:, :], wt["WAC1D"][:, :],
                                 S["SG"][0:32, :], start=False, stop=False)
                nc.tensor.matmul(S["ACC"][:, :], wt["WAC1P"][:, :],
                                 S["SP"][0:32, :], start=False, stop=True)
                OUT = sp.tile([8, NL], F32, tag=f"OUT{c}")
                nc.scalar.activation(OUT[:, :], S["ACC"][:, :], AF.Copy,
                                     bias=float(init_c))
                nc.sync.dma_start(out_d[c].ap(), OUT[:, :])
    ctx.close()
    nc.compile()
    return nc


def _pack_w(w_core):
    """w_core [512, 2, 2047] f32 -> {(c,q): [32, 256*32] f32}."""
    out = {}
    T2 = NQ * CH
    for c in range(NCH):
        wc = w_core[c * LCH:(c + 1) * LCH].reshape(NG, NL, 2, TR)
        arr = np.zeros((32, T2, NL), np.float32)
        for g in range(NG):
            arr[g, :TR, :] = wc[g, :, 0, :].T         # x1 slot
            arr[8 + g, :TR, :] = wc[g, :, 1, :].T     # x2 slot
        for q in range(NQ):
            out[(c, q)] = np.ascontiguousarray(
                arr[:, q * CH:(q + 1) * CH, :]).reshape(
                    32, CH * NL).astype(ml_dtypes.bfloat16)
    return out


_PROG_CACHE = {}


def kernel(w, K, L, M, Mo):
    w = np.asarray(w, np.float32)
    K = np.asarray(K, np.float32)
    L = np.asarray(L, np.float32)
    M = np.asarray(M, np.float32)
    Mo = np.asarray(Mo, np.float32)
    B = w.shape[0]
    Wmats, Mo_f, init_c = _build_weights(K, L, M, Mo)

    key = (w.shape, K.tobytes(), L.tobytes(), M.tobytes(), Mo.tobytes())
    if key not in _PROG_CACHE:
        _PROG_CACHE[key] = _build_program(Mo_f, init_c)
    nc = _PROG_CACHE[key]

    xa, xb, gi = _init_consts()
    in_maps = []
    for core in range(N_CORES):
        m = {n: np.asarray(Wmats[n]) for n in Wmats}
        m["XIA"], m["XIB"], m["GI"] = xa, xb, gi
        wp = _pack_w(w[core * LPC:(core + 1) * LPC])
        for (c, q), arr in wp.items():
            m[f"w{c}_{q}"] = arr
        in_maps.append(m)

    kw = {}
    if os.environ.get("KERNEL_TRACE"):
        kw = dict(trace=True)
        if os.environ.get("KERNEL_TRACE_DIR"):
            kw["tmpdir"] = os.environ["KERNEL_TRACE_DIR"]
    res = bass_utils.run_bass_kernel_spmd(nc, in_maps,
                                          core_ids=list(range(N_CORES)), **kw)
    globals()["_LAST_RES"] = res
    out = np.empty(B, np.float32)
    for core in range(N_CORES):
        for c in range(NCH):
            o = res.results[core][f"out{c}"]       # [8, 32]
            lo = core * LPC + c * LCH
            out[lo:lo + LCH] = o.reshape(LCH)
    return out


# revision 36
# speedup vs baseline: 1.1594x; 1.1594x over previous
"""Trainium2 Bass kernel for nn_CSTR: B=4096-lane vmapped 2047-step rollout.

v3: data-parallel over 8 cores (512 lanes each), 2 independent 256-lane
chains per core, emission-staggered half a step apart so their
instruction streams fill each other's dependency gaps.

Per chain: slot-major layout, slot = 8 partitions x 32 free lanes,
state slots (x1, x2, xh1, xh2). Mega input tile X[104,32] bf16 =
[state@0; tanh@32; tanh^2@64; ones@96] (every engine-written sub-view
32-partition aligned). The full state update collapses to ONE bf16
matmul producing the O(H) increment Delta (u = K@xhat composed in, RK4
constants on the ones slot); the exact fp32 state G[32,32] is kept via
a vector add G' = Delta + (G + w). Gating: W_VD -> [Ls@rx + M;
0pad+(x-fp); Qc@x], products on DVE, W_PH -> phi (x4 dup), sigmoid on
ScalarE; the blend add uses a zero-padded delta*(x-fp) [32,32] so both
the fp32 state and bf16 mirror update with one aligned add each.
Stage cost accumulates into a persistent PSUM bank every 4 steps from
two [128,32] staging tiles (deltas, Qc products). Only O(H)-scaled
terms flow through bf16 matmuls; fp32 trajectory exactness is
preserved via the vector adds (2.7e-4 max rel err vs reference).
"""
import os
import sys
import numpy as np
import ml_dtypes
from contextlib import ExitStack

sys.path.insert(0, "/opt/trn_rl_repo")

import concourse.bacc as bacc
import concourse.bass as bass
import concourse.mybir as mybir
import concourse.tile as tile
from concourse import bass_utils

F32 = mybir.dt.float32
BF16 = mybir.dt.bfloat16
AF = mybir.ActivationFunctionType

H = np.float32(0.01)
LAM = np.float32(1.0)
B_TOT, N_CORES = 4096, 8
LPC = 512                 # lanes per core
NCH = 2                   # chains per core
LCH = 256                 # lanes per chain
NG, NL = 8, 32            # groups x free lanes per chain
TR = 2047                 # real steps
CH = 256                  # steps per w chunk
NQ = 8                    # chunks (last holds 255 real steps)

C1 = np.float32(1.0) - H
GC = np.float32(0.5) * H
EC = np.float32(H * H / 2)
A0 = np.float32(5e-5)
ALPHA = np.float32(H - np.float32(1e-6) / 3)


def _kron8(A):
    """lhsT for slot map A[out_slot, in_slot] -> [8*in, 8*out] bf16."""
    k = np.kron(np.ascontiguousarray(A.T), np.eye(NG, dtype=np.float32))
    return k.astype(ml_dtypes.bfloat16)


def _build_weights(K, L, M, Mo):
    K1, K2 = np.float32(K[0, 0]), np.float32(K[0, 1])
    Ls = ((L + L.T) * np.float32(0.5)).astype(np.float32)
    Mv = M[0].astype(np.float32)
    Qc = np.array([[1 + K1 * K1, K1 * K2], [K1 * K2, 1 + K2 * K2]], np.float32)
    # slot order (x1, x2, xh1, xh2) == rx order (x1, x2, fp1, fp2)

    # W_X inputs: state(4) T(4) Q(4) ones(1) = 13 slots -> Delta(4)
    uc = np.array([H, GC, H, GC], np.float32)
    Kv = np.array([0, 0, K1, K2], np.float32)
    A_S = (C1 - 1) * np.eye(4, dtype=np.float32) + np.outer(uc, Kv)
    A_T = np.zeros((4, 4), np.float32)
    A_T[0, 1] = ALPHA; A_T[1, 0] = -H; A_T[2, 3] = ALPHA; A_T[3, 2] = -H
    A_Q = np.zeros((4, 4), np.float32)
    A_Q[0, 1] = -A0; A_Q[2, 3] = -A0
    A_ONES = np.array([A0, -EC, A0, -EC], np.float32).reshape(4, 1)
    WX = np.concatenate([A_S, A_T, A_Q, A_ONES], axis=1)   # [4, 13]

    # W_VD: 13 input slots -> [y(4); 0pad(2); DIF(2); y2(2)] = 10 out,
    # composed with the state update so it reads the PRE-update X tile:
    # VD = (Avd + AV4@A_WX)@X_old + AV4@w  (exact linear algebra).
    Avd = np.zeros((10, 13), np.float32)
    for s in range(4):
        for sp in range(4):
            Avd[s, sp] = Ls[s, sp]
        Avd[s, 12] = Mv[s]
    Avd[6, 0] = 1; Avd[6, 2] = -1          # DIF1 = x1 - xh1
    Avd[7, 1] = 1; Avd[7, 3] = -1          # DIF2 = x2 - xh2
    Avd[8, 0] = Qc[0, 0]; Avd[8, 1] = Qc[0, 1]
    Avd[9, 0] = Qc[1, 0]; Avd[9, 1] = Qc[1, 1]
    AV4 = Avd[:, 0:4]
    WVDX = Avd + AV4 @ WX                  # [10, 13]
    WVDW = AV4                             # [10, 4] applied to (w1,w2,0,0)
    # single fused matmul: out slots [VD(10); pad(2); Delta(4)] = 16
    WBIG = np.concatenate(
        [WVDX, np.zeros((2, 13), np.float32), WX], axis=0)   # [16, 13]

    APH = np.ones((4, 4), np.float32)      # phi, x4 dup
    AC4D = np.zeros((1, 16), np.float32)
    AC4P = np.zeros((1, 16), np.float32)
    for b in range(4):
        AC4D[0, 4 * b] = LAM
        AC4P[0, 4 * b] = 1; AC4P[0, 4 * b + 1] = 1
    AC1D = np.zeros((1, 4), np.float32); AC1D[0, 0] = LAM
    AC1P = np.zeros((1, 4), np.float32); AC1P[0, 0] = 1; AC1P[0, 1] = 1
    AFIN = np.array([[10.0, 10.0]], np.float32)

    W = {
        "WX": _kron8(WX),        # [104, 32]
        "WVD": _kron8(WVDX),     # [104, 80]
        "WVDW": _kron8(WVDW),    # [32, 80]
        "WPH": _kron8(APH),      # [32, 32]
        "WAC4D": _kron8(AC4D),   # [128, 8]
        "WAC4P": _kron8(AC4P),   # [128, 8]
        "WAC1D": _kron8(AC1D),   # [32, 8]
        "WAC1P": _kron8(AC1P),   # [32, 8]
        "WFIN": _kron8(AFIN),    # [16, 8]
    }
    init_c = float(1.0 + K1 * K1 + LAM)
    return W, float(Mo[0, 0]), init_c


WSHAPES = {"WX": (104, 32), "WVD": (104, 80), "WVDW": (32, 80),
           "WPH": (32, 32),
           "WAC4D": (128, 8), "WAC4P": (128, 8), "WAC1D": (32, 8),
           "WAC1P": (32, 8), "WFIN": (16, 8)}


def _init_consts():
    xa = np.zeros((104, NL), np.float32)
    for lo, v in ((0, 1.0), (8, 0.0), (16, 1.0), (24, 0.0)):   # state
        xa[lo:lo + 8] = v
    xa[96:104] = 1.0                                           # ones
    xb = np.zeros((104, NL), np.float32)
    xb[96:104] = 1.0
    gi = np.zeros((32, NL), np.float32)
    gi[0:8] = 1.0; gi[16:24] = 1.0
    return (xa.astype(ml_dtypes.bfloat16), xb.astype(ml_dtypes.bfloat16), gi)


def _build_program(Mo_f, init_c):
    nc = bacc.Bacc("TRN2", target_bir_lowering=False, debug=False)

    wd = {}
    for c in range(NCH):
        for q in range(NQ):
            wd[(c, q)] = nc.dram_tensor(
                f"w{c}_{q}", [32, CH * NL], BF16, kind="ExternalInput")
    wdram = {n: nc.dram_tensor(n, list(WSHAPES[n]), BF16, kind="ExternalInput")
             for n in WSHAPES}
    xa_d = nc.dram_tensor("XIA", [104, NL], BF16, kind="ExternalInput")
    xb_d = nc.dram_tensor("XIB", [104, NL], BF16, kind="ExternalInput")
    gi_d = nc.dram_tensor("GI", [32, NL], F32, kind="ExternalInput")
    out_d = [nc.dram_tensor(f"out{c}", [NG, NL], F32, kind="ExternalOutput")
             for c in range(NCH)]

    ctx = ExitStack()
    with tile.TileContext(nc) as tc:
        with tc.tile_pool(name="wq", bufs=2) as wpool, \
             tc.tile_pool(name="wt", bufs=1) as cpool, \
             tc.tile_pool(name="sb", bufs=3) as sp, \
             tc.tile_pool(name="ps", bufs=1, space="PSUM") as pp, \
             tc.tile_pool(name="pacc", bufs=1, space="PSUM") as pa:

            wt = {}
            for n in WSHAPES:
                wt[n] = cpool.tile(list(WSHAPES[n]), BF16, tag=n, name=n + "_t")
                nc.sync.dma_start(wt[n][:, :], wdram[n].ap())

            wtiles = {}

            def get_wtile(c, q):
                if (c, q) not in wtiles:
                    t = wpool.tile([32, CH * NL], BF16, tag=f"w{c}",
                                   name=f"w{c}_{q}_t")
                    nc.sync.dma_start(t[:, :], wd[(c, q)].ap())
                    wtiles[(c, q)] = t
                return wtiles[(c, q)]

            chs = []
            for c in range(NCH):
                S = {}
                S["X"] = [cpool.tile([104, NL], BF16, tag=f"XA{c}", name=f"XA{c}"),
                          cpool.tile([104, NL], BF16, tag=f"XB{c}", name=f"XB{c}")]
                S["G"] = [cpool.tile([32, NL], F32, tag=f"GA{c}", name=f"GA{c}"),
                          cpool.tile([32, NL], F32, tag=f"GB{c}", name=f"GB{c}")]
                S["RW"] = cpool.tile([32, NL], F32, tag=f"RW{c}", name=f"RW{c}")
                S["SG"] = cpool.tile([128, NL], BF16, tag=f"SG{c}", name=f"SG{c}")
                S["SP"] = cpool.tile([128, NL], BF16, tag=f"SP{c}", name=f"SP{c}")
                S["MOB"] = cpool.tile([32, 1], F32, tag=f"MOB{c}", name=f"MOB{c}")
                S["ACC"] = pa.tile([8, NL], F32, tag=f"ACC{c}", name=f"ACC{c}")
                S["tagD"] = f"Dp{c}"
                S["tagV"] = f"VDp{c}"
                S["tagP"] = f"PHp{c}"
                S["tagPP"] = f"PP{c}"
                S["tagM"] = f"M1{c}"
                chs.append(S)

            Z = cpool.tile([32, NL], BF16, tag="Z", name="Z")
            nc.vector.memset(Z[:, :], 0.0)

            for c, S in enumerate(chs):
                nc.sync.dma_start(S["X"][0][:, :], xa_d.ap())
                nc.sync.dma_start(S["X"][1][:, :], xb_d.ap())
                nc.sync.dma_start(S["G"][0][:, :], gi_d.ap())
                nc.vector.memset(S["SG"][:, :], 0.0)
                nc.gpsimd.memset(S["SP"][:, :], 0.0)
                nc.vector.memset(S["MOB"][:, :], Mo_f)
                nc.tensor.matmul(S["ACC"][:, :], wt["WAC1D"][:, :], Z[:, :],
                                 start=True, stop=False)
                w0 = get_wtile(c, 0)
                nc.vector.tensor_add(S["RW"][:, :], S["G"][0][:, :],
                                     w0[:, 0:NL])
                if c == 1:
                    # one-time ~1.8us delay on chain B's first state add:
                    # forces the two chains into anti-phase so their ops
                    # fill each other's dependency gaps instead of
                    # contending in lockstep.
                    for _ in range(10):
                        nc.vector.tensor_add(S["RW"][:, :], S["RW"][:, :],
                                             Z[:, :])
                get_wtile(c, 1)

            def step_ops(S, p, wv_cur, wv_next, b, do_gate, do_cost, do_acc):
                """One step of one chain; p = input parity, b = SG block."""
                Xi, Xo = S["X"][p], S["X"][1 - p]
                Go = S["G"][1 - p]
                RW, SG, SP = S["RW"], S["SG"], S["SP"]
                Dp = pp.tile([32, NL], F32, tag=S["tagD"])
                ops = []
                ops.append(lambda: nc.scalar.activation(
                    Xi[32:64, :], Xi[0:32, :], AF.Tanh))
                ops.append(lambda: nc.scalar.activation(
                    Xi[64:96, :], Xi[32:64, :], AF.Square))
                ops.append(lambda: nc.tensor.matmul(
                    Dp[:, :], wt["WX"][:, :], Xi[:, :],
                    start=True, stop=True))
                if do_gate:
                    VDp = pp.tile([80, NL], F32, tag=S["tagV"])
                    ops.append(lambda: nc.tensor.matmul(
                        VDp[:, :], wt["WVD"][:, :], Xi[:, :],
                        start=True, stop=False))
                    ops.append(lambda: nc.tensor.matmul(
                        VDp[:, :], wt["WVDW"][:, :], wv_cur,
                        start=False, stop=True))
                ops.append(lambda: nc.vector.tensor_add(
                    Xo[0:32, :], Dp[:, :], RW[:, :]))
                ops.append(lambda: nc.vector.tensor_add(
                    Go[:, :], Dp[:, :], RW[:, :]))
                if do_gate:
                    PHp = pp.tile([32, NL], F32, tag=S["tagP"])
                    PP = sp.tile([32, NL], BF16, tag=S["tagPP"])
                    M1 = sp.tile([32, NL], F32, tag=S["tagM"])
                    ops.append(lambda: nc.vector.tensor_mul(
                        PP[:, :], Xo[0:32, :], VDp[0:32, :]))
                    if do_cost:
                        ops.append(lambda: nc.vector.tensor_mul(
                            SP[32 * b:32 * b + 16, :],
                            Xo[0:16, :], VDp[64:80, :]))
                    ops.append(lambda: nc.tensor.matmul(
                        PHp[:, :], wt["WPH"][:, :], PP[:, :],
                        start=True, stop=True))
                    ops.append(lambda: nc.scalar.activation(
                        SG[32 * b:32 * b + 32, :], PHp[:, :], AF.Sigmoid,
                        bias=S["MOB"][:, :]))
                    ops.append(lambda: nc.vector.tensor_mul(
                        M1[:, :], SG[32 * b:32 * b + 32, :], VDp[32:64, :]))
                    ops.append(lambda: nc.vector.tensor_add(
                        Xo[0:32, :], Xo[0:32, :], M1[:, :]))
                    ops.append(lambda: nc.gpsimd.tensor_add(
                        Go[:, :], Go[:, :], M1[:, :]))
                    if do_acc:
                        ops.append(lambda: nc.tensor.matmul(
                            S["ACC"][:, :], wt["WAC4D"][:, :], SG[:, :],
                            start=False, stop=False))
                        ops.append(lambda: nc.tensor.matmul(
                            S["ACC"][:, :], wt["WAC4P"][:, :], SP[:, :],
                            start=False, stop=False))
                if wv_next is not None:
                    ops.append(lambda: nc.gpsimd.tensor_add(
                        RW[:, :], Go[:, :], wv_next))
                return ops

            def emit_group(k0, wts, wts_next_chunk):
                """4 steps (k0..k0+3) for both chains, B staggered."""
                for j in range(4):
                    k = k0 + j
                    do_gate = k <= TR - 2
                    do_cost = k <= TR - 3
                    do_acc = do_cost and (j == 3)
                    opsl = []
                    for c, S in enumerate(chs):
                        wtile, base = wts[c]
                        if isinstance(base, int):
                            wv_cur = wtile[:, base + j * NL:base + (j + 1) * NL]
                        else:
                            wv_cur = wtile[:, bass.ds(base + j * NL, NL)]
                        if k + 1 <= TR - 1:
                            if j == 3 and wts_next_chunk is not None:
                                nwtile, nbase = wts_next_chunk[c]
                                wv_next = nwtile[:, nbase:nbase + NL]
                            elif isinstance(base, int):
                                nb = base + (j + 1) * NL
                                wv_next = wtile[:, nb:nb + NL]
                            else:
                                wv_next = wtile[:, bass.ds(base + (j + 1) * NL, NL)]
                        else:
                            wv_next = None
                        opsl.append(step_ops(S, j % 2, wv_cur, wv_next, j,
                                             do_gate, do_cost, do_acc))
                    sa, sb = opsl
                    off = 8   # stagger chain B ~half a step behind A
                    for i in range(max(len(sa), len(sb)) + off):
                        if i < len(sa):
                            sa[i]()
                        if 0 <= i - off < len(sb):
                            sb[i - off]()

            # chunks of 256 steps: 31 hw-loop bodies of 8 steps (248) +
            # 8 static tail steps (2 groups).
            for q in range(NQ):
                for c in range(NCH):
                    get_wtile(c, q)
                with tc.For_i(0, 31, 1) as iv:
                    wts = [(wtiles[(c, q)], iv * (8 * NL)) for c in range(NCH)]
                    emit_group(q * CH, wts, None)
                    wts2 = [(wtiles[(c, q)], iv * (8 * NL) + 4 * NL)
                            for c in range(NCH)]
                    emit_group(q * CH, wts2, None)
                nxt = None
                if q + 1 < NQ:
                    nxt = [(get_wtile(c, q + 1), 0) for c in range(NCH)]
                wts = [(wtiles[(c, q)], 248 * NL) for c in range(NCH)]
                emit_group(q * CH + 248, wts, None)
                wts = [(wtiles[(c, q)], 252 * NL) for c in range(NCH)]
                emit_group(q * CH + 252, wts, nxt)

            # epilogue: after 2047 steps state parity lands in G[1]
            for c, S in enumerate(chs):
                Gl = S["G"][1]
                FSQ = sp.tile([16, NL], BF16, tag=f"FSQ{c}")
                nc.vector.tensor_mul(FSQ[:, :], Gl[0:16, :], Gl[0:16, :])
                nc.tensor.matmul(S["ACC"][:, :], wt["WFIN"][:, :], FSQ[:, :],
                                 start=False, stop=False)
                nc.tensor.matmul(S["ACC"][:, :], wt["WAC1D"][:, :],
                                 S["SG"][0:32, :], start=False, stop=False)
                nc.tensor.matmul(S["ACC"][:, :], wt["WAC1P"][:, :],
                                 S["SP"][0:32, :], start=False, stop=True)
                OUT = sp.tile([8, NL], F32, tag=f"OUT{c}")
                nc.scalar.activation(OUT[:, :], S["ACC"][:, :], AF.Copy,
                                     bias=float(init_c))
                nc.sync.dma_start(out_d[c].ap(), OUT[:, :])
    ctx.close()
    nc.compile()
    return nc


def _pack_w(w_core):
    """w_core [512, 2, 2047] f32 -> {(c,q): [32, 256*32] f32}."""
    out = {}
    T2 = NQ * CH
    for c in range(NCH):
        wc = w_core[c * LCH:(c + 1) * LCH].reshape(NG, NL, 2, TR)
        arr = np.zeros((32, T2, NL), np.float32)
        for g in range(NG):
            arr[g, :TR, :] = wc[g, :, 0, :].T         # x1 slot
            arr[8 + g, :TR, :] = wc[g, :, 1, :].T     # x2 slot
        for q in range(NQ):
            out[(c, q)] = np.ascontiguousarray(
                arr[:, q * CH:(q + 1) * CH, :]).reshape(
                    32, CH * NL).astype(ml_dtypes.bfloat16)
    return out


_PROG_CACHE = {}


def kernel(w, K, L, M, Mo):
    w = np.asarray(w, np.float32)
    K = np.asarray(K, np.float32)
    L = np.asarray(L, np.float32)
    M = np.asarray(M, np.float32)
    Mo = np.asarray(Mo, np.float32)
    B = w.shape[0]
    Wmats, Mo_f, init_c = _build_weights(K, L, M, Mo)

    key = (w.shape, K.tobytes(), L.tobytes(), M.tobytes(), Mo.tobytes())
    if key not in _PROG_CACHE:
        _PROG_CACHE[key] = _build_program(Mo_f, init_c)
    nc = _PROG_CACHE[key]

    xa, xb, gi = _init_consts()
    in_maps = []
    for core in range(N_CORES):
        m = {n: np.asarray(Wmats[n]) for n in Wmats}
        m["XIA"], m["XIB"], m["GI"] = xa, xb, gi
        wp = _pack_w(w[core * LPC:(core + 1) * LPC])
        for (c, q), arr in wp.items():
            m[f"w{c}_{q}"] = arr
        in_maps.append(m)

    kw = {}
    if os.environ.get("KERNEL_TRACE"):
        kw = dict(trace=True)
        if os.environ.get("KERNEL_TRACE_DIR"):
            kw["tmpdir"] = os.environ["KERNEL_TRACE_DIR"]
    res = bass_utils.run_bass_kernel_spmd(nc, in_maps,
                                          core_ids=list(range(N_CORES)), **kw)
    globals()["_LAST_RES"] = res
    out = np.empty(B, np.float32)
    for core in range(N_CORES):
        for c in range(NCH):
            o = res.results[core][f"out{c}"]       # [8, 32]
            lo = core * LPC + c * LCH
            out[lo:lo + LCH] = o.reshape(LCH)
    return out


# revision 37
# speedup vs baseline: 1.1681x; 1.0075x over previous
"""Trainium2 Bass kernel for nn_CSTR: B=4096-lane vmapped 2047-step rollout.

v3: data-parallel over 8 cores (512 lanes each), 2 independent 256-lane
chains per core, emission-staggered half a step apart so their
instruction streams fill each other's dependency gaps.

Per chain: slot-major layout, slot = 8 partitions x 32 free lanes,
state slots (x1, x2, xh1, xh2). Mega input tile X[104,32] bf16 =
[state@0; tanh@32; tanh^2@64; ones@96] (every engine-written sub-view
32-partition aligned). The full state update collapses to ONE bf16
matmul producing the O(H) increment Delta (u = K@xhat composed in, RK4
constants on the ones slot); the exact fp32 state G[32,32] is kept via
a vector add G' = Delta + (G + w). Gating: W_VD -> [Ls@rx + M;
0pad+(x-fp); Qc@x], products on DVE, W_PH -> phi (x4 dup), sigmoid on
ScalarE; the blend add uses a zero-padded delta*(x-fp) [32,32] so both
the fp32 state and bf16 mirror update with one aligned add each.
Stage cost accumulates into a persistent PSUM bank every 4 steps from
two [128,32] staging tiles (deltas, Qc products). Only O(H)-scaled
terms flow through bf16 matmuls; fp32 trajectory exactness is
preserved via the vector adds (2.7e-4 max rel err vs reference).
"""
import os
import sys
import numpy as np
import ml_dtypes
from contextlib import ExitStack

sys.path.insert(0, "/opt/trn_rl_repo")

import concourse.bacc as bacc
import concourse.bass as bass
import concourse.mybir as mybir
import concourse.tile as tile
from concourse import bass_utils

F32 = mybir.dt.float32
BF16 = mybir.dt.bfloat16
AF = mybir.ActivationFunctionType

H = np.float32(0.01)
LAM = np.float32(1.0)
B_TOT, N_CORES = 4096, 8
LPC = 512                 # lanes per core
NCH = 2                   # chains per core
LCH = 256                 # lanes per chain
NG, NL = 8, 32            # groups x free lanes per chain
TR = 2047                 # real steps
CH = 256                  # steps per w chunk
NQ = 8                    # chunks (last holds 255 real steps)

C1 = np.float32(1.0) - H
GC = np.float32(0.5) * H
EC = np.float32(H * H / 2)
A0 = np.float32(5e-5)
ALPHA = np.float32(H - np.float32(1e-6) / 3)


def _kron8(A):
    """lhsT for slot map A[out_slot, in_slot] -> [8*in, 8*out] bf16."""
    k = np.kron(np.ascontiguousarray(A.T), np.eye(NG, dtype=np.float32))
    return k.astype(ml_dtypes.bfloat16)


def _build_weights(K, L, M, Mo):
    K1, K2 = np.float32(K[0, 0]), np.float32(K[0, 1])
    Ls = ((L + L.T) * np.float32(0.5)).astype(np.float32)
    Mv = M[0].astype(np.float32)
    Qc = np.array([[1 + K1 * K1, K1 * K2], [K1 * K2, 1 + K2 * K2]], np.float32)
    # slot order (x1, x2, xh1, xh2) == rx order (x1, x2, fp1, fp2)

    # W_X inputs: state(4) T(4) Q(4) ones(1) = 13 slots -> Delta(4)
    uc = np.array([H, GC, H, GC], np.float32)
    Kv = np.array([0, 0, K1, K2], np.float32)
    A_S = (C1 - 1) * np.eye(4, dtype=np.float32) + np.outer(uc, Kv)
    A_T = np.zeros((4, 4), np.float32)
    A_T[0, 1] = ALPHA; A_T[1, 0] = -H; A_T[2, 3] = ALPHA; A_T[3, 2] = -H
    A_Q = np.zeros((4, 4), np.float32)
    A_Q[0, 1] = -A0; A_Q[2, 3] = -A0
    A_ONES = np.array([A0, -EC, A0, -EC], np.float32).reshape(4, 1)
    WX = np.concatenate([A_S, A_T, A_Q, A_ONES], axis=1)   # [4, 13]

    # W_VD: 13 input slots -> [y(4); 0pad(2); DIF(2); y2(2)] = 10 out,
    # composed with the state update so it reads the PRE-update X tile:
    # VD = (Avd + AV4@A_WX)@X_old + AV4@w  (exact linear algebra).
    Avd = np.zeros((10, 13), np.float32)
    for s in range(4):
        for sp in range(4):
            Avd[s, sp] = Ls[s, sp]
        Avd[s, 12] = Mv[s]
    Avd[6, 0] = 1; Avd[6, 2] = -1          # DIF1 = x1 - xh1
    Avd[7, 1] = 1; Avd[7, 3] = -1          # DIF2 = x2 - xh2
    Avd[8, 0] = Qc[0, 0]; Avd[8, 1] = Qc[0, 1]
    Avd[9, 0] = Qc[1, 0]; Avd[9, 1] = Qc[1, 1]
    AV4 = Avd[:, 0:4]
    WVDX = Avd + AV4 @ WX                  # [10, 13]
    WVDW = AV4                             # [10, 4] applied to (w1,w2,0,0)
    # single fused matmul: out slots [VD(10); pad(2); Delta(4)] = 16
    WBIG = np.concatenate(
        [WVDX, np.zeros((2, 13), np.float32), WX], axis=0)   # [16, 13]

    APH = np.ones((4, 4), np.float32)      # phi, x4 dup
    AC4D = np.zeros((1, 16), np.float32)
    AC4P = np.zeros((1, 16), np.float32)
    for b in range(4):
        AC4D[0, 4 * b] = LAM
        AC4P[0, 4 * b] = 1; AC4P[0, 4 * b + 1] = 1
    AC1D = np.zeros((1, 4), np.float32); AC1D[0, 0] = LAM
    AC1P = np.zeros((1, 4), np.float32); AC1P[0, 0] = 1; AC1P[0, 1] = 1
    AFIN = np.array([[10.0, 10.0]], np.float32)

    W = {
        "WX": _kron8(WX),        # [104, 32]
        "WVD": _kron8(WVDX),     # [104, 80]
        "WVDW": _kron8(WVDW),    # [32, 80]
        "WPH": _kron8(APH),      # [32, 32]
        "WAC4D": _kron8(AC4D),   # [128, 8]
        "WAC4P": _kron8(AC4P),   # [128, 8]
        "WAC1D": _kron8(AC1D),   # [32, 8]
        "WAC1P": _kron8(AC1P),   # [32, 8]
        "WFIN": _kron8(AFIN),    # [16, 8]
    }
    init_c = float(1.0 + K1 * K1 + LAM)
    return W, float(Mo[0, 0]), init_c


WSHAPES = {"WX": (104, 32), "WVD": (104, 80), "WVDW": (32, 80),
           "WPH": (32, 32),
           "WAC4D": (128, 8), "WAC4P": (128, 8), "WAC1D": (32, 8),
           "WAC1P": (32, 8), "WFIN": (16, 8)}


def _init_consts():
    xa = np.zeros((104, NL), np.float32)
    for lo, v in ((0, 1.0), (8, 0.0), (16, 1.0), (24, 0.0)):   # state
        xa[lo:lo + 8] = v
    xa[96:104] = 1.0                                           # ones
    xb = np.zeros((104, NL), np.float32)
    xb[96:104] = 1.0
    gi = np.zeros((32, NL), np.float32)
    gi[0:8] = 1.0; gi[16:24] = 1.0
    return (xa.astype(ml_dtypes.bfloat16), xb.astype(ml_dtypes.bfloat16), gi)


def _build_program(Mo_f, init_c):
    nc = bacc.Bacc("TRN2", target_bir_lowering=False, debug=False)

    wd = {}
    for c in range(NCH):
        for q in range(NQ):
            wd[(c, q)] = nc.dram_tensor(
                f"w{c}_{q}", [32, CH * NL], BF16, kind="ExternalInput")
    wdram = {n: nc.dram_tensor(n, list(WSHAPES[n]), BF16, kind="ExternalInput")
             for n in WSHAPES}
    xa_d = nc.dram_tensor("XIA", [104, NL], BF16, kind="ExternalInput")
    xb_d = nc.dram_tensor("XIB", [104, NL], BF16, kind="ExternalInput")
    gi_d = nc.dram_tensor("GI", [32, NL], F32, kind="ExternalInput")
    out_d = [nc.dram_tensor(f"out{c}", [NG, NL], F32, kind="ExternalOutput")
             for c in range(NCH)]

    ctx = ExitStack()
    with tile.TileContext(nc) as tc:
        with tc.tile_pool(name="wq", bufs=2) as wpool, \
             tc.tile_pool(name="wt", bufs=1) as cpool, \
             tc.tile_pool(name="sb", bufs=3) as sp, \
             tc.tile_pool(name="ps", bufs=1, space="PSUM") as pp, \
             tc.tile_pool(name="pacc", bufs=1, space="PSUM") as pa:

            wt = {}
            for n in WSHAPES:
                wt[n] = cpool.tile(list(WSHAPES[n]), BF16, tag=n, name=n + "_t")
                nc.sync.dma_start(wt[n][:, :], wdram[n].ap())

            wtiles = {}

            def get_wtile(c, q):
                if (c, q) not in wtiles:
                    t = wpool.tile([32, CH * NL], BF16, tag=f"w{c}",
                                   name=f"w{c}_{q}_t")
                    nc.sync.dma_start(t[:, :], wd[(c, q)].ap())
                    wtiles[(c, q)] = t
                return wtiles[(c, q)]

            chs = []
            for c in range(NCH):
                S = {}
                S["X"] = [cpool.tile([104, NL], BF16, tag=f"XA{c}", name=f"XA{c}"),
                          cpool.tile([104, NL], BF16, tag=f"XB{c}", name=f"XB{c}")]
                S["G"] = [cpool.tile([32, NL], F32, tag=f"GA{c}", name=f"GA{c}"),
                          cpool.tile([32, NL], F32, tag=f"GB{c}", name=f"GB{c}")]
                S["RW"] = cpool.tile([32, NL], F32, tag=f"RW{c}", name=f"RW{c}")
                S["SG"] = cpool.tile([128, NL], BF16, tag=f"SG{c}", name=f"SG{c}")
                S["SP"] = cpool.tile([128, NL], BF16, tag=f"SP{c}", name=f"SP{c}")
                S["MOB"] = cpool.tile([32, 1], F32, tag=f"MOB{c}", name=f"MOB{c}")
                S["ACC"] = pa.tile([8, NL], F32, tag=f"ACC{c}", name=f"ACC{c}")
                S["tagD"] = f"Dp{c}"
                S["tagV"] = f"VDp{c}"
                S["tagP"] = f"PHp{c}"
                S["tagPP"] = f"PP{c}"
                S["tagM"] = f"M1{c}"
                chs.append(S)

            Z = cpool.tile([32, NL], BF16, tag="Z", name="Z")
            nc.vector.memset(Z[:, :], 0.0)

            for c, S in enumerate(chs):
                nc.sync.dma_start(S["X"][0][:, :], xa_d.ap())
                nc.sync.dma_start(S["X"][1][:, :], xb_d.ap())
                nc.sync.dma_start(S["G"][0][:, :], gi_d.ap())
                nc.vector.memset(S["SG"][:, :], 0.0)
                nc.gpsimd.memset(S["SP"][:, :], 0.0)
                nc.vector.memset(S["MOB"][:, :], Mo_f)
                nc.tensor.matmul(S["ACC"][:, :], wt["WAC1D"][:, :], Z[:, :],
                                 start=True, stop=False)
                w0 = get_wtile(c, 0)
                nc.vector.tensor_add(S["RW"][:, :], S["G"][0][:, :],
                                     w0[:, 0:NL])
                if c == 1:
                    # one-time ~1.8us delay on chain B's first state add:
                    # forces the two chains into anti-phase so their ops
                    # fill each other's dependency gaps instead of
                    # contending in lockstep.
                    for _ in range(10):
                        nc.vector.tensor_add(S["RW"][:, :], S["RW"][:, :],
                                             Z[:, :])
                get_wtile(c, 1)

            def step_ops(S, p, wv_cur, wv_next, b, do_gate, do_cost, do_acc):
                """One step of one chain; p = input parity, b = SG block."""
                Xi, Xo = S["X"][p], S["X"][1 - p]
                Go = S["G"][1 - p]
                RW, SG, SP = S["RW"], S["SG"], S["SP"]
                Dp = pp.tile([32, NL], F32, tag=S["tagD"])
                ops = []
                ops.append(lambda: nc.scalar.activation(
                    Xi[32:64, :], Xi[0:32, :], AF.Tanh))
                ops.append(lambda: nc.scalar.activation(
                    Xi[64:96, :], Xi[32:64, :], AF.Square))
                ops.append(lambda: nc.tensor.matmul(
                    Dp[:, :], wt["WX"][:, :], Xi[:, :],
                    start=True, stop=True))
                if do_gate:
                    VDp = pp.tile([80, NL], F32, tag=S["tagV"])
                    ops.append(lambda: nc.tensor.matmul(
                        VDp[:, :], wt["WVD"][:, :], Xi[:, :],
                        start=True, stop=False))
                    ops.append(lambda: nc.tensor.matmul(
                        VDp[:, :], wt["WVDW"][:, :], wv_cur,
                        start=False, stop=True))
                ops.append(lambda: nc.vector.tensor_add(
                    Xo[0:32, :], Dp[:, :], RW[:, :]))
                ops.append(lambda: nc.vector.tensor_add(
                    Go[:, :], Dp[:, :], RW[:, :]))
                if do_gate:
                    PHp = pp.tile([32, NL], F32, tag=S["tagP"])
                    PP = sp.tile([32, NL], BF16, tag=S["tagPP"])
                    M1 = sp.tile([32, NL], F32, tag=S["tagM"])
                    ops.append(lambda: nc.vector.tensor_mul(
                        PP[:, :], Xo[0:32, :], VDp[0:32, :]))
                    if do_cost:
                        ops.append(lambda: nc.vector.tensor_mul(
                            SP[32 * b:32 * b + 16, :],
                            Xo[0:16, :], VDp[64:80, :]))
                    ops.append(lambda: nc.tensor.matmul(
                        PHp[:, :], wt["WPH"][:, :], PP[:, :],
                        start=True, stop=True))
                    ops.append(lambda: nc.scalar.activation(
                        SG[32 * b:32 * b + 32, :], PHp[:, :], AF.Sigmoid,
                        bias=S["MOB"][:, :]))
                    ops.append(lambda: nc.vector.tensor_mul(
                        M1[:, :], SG[32 * b:32 * b + 32, :], VDp[32:64, :]))
                    ops.append(lambda: nc.vector.tensor_add(
                        Xo[0:32, :], Xo[0:32, :], M1[:, :]))
                    ops.append(lambda: nc.gpsimd.tensor_add(
                        Go[:, :], Go[:, :], M1[:, :]))
                    if do_acc:
                        ops.append(lambda: nc.tensor.matmul(
                            S["ACC"][:, :], wt["WAC4D"][:, :], SG[:, :],
                            start=False, stop=False))
                        ops.append(lambda: nc.tensor.matmul(
                            S["ACC"][:, :], wt["WAC4P"][:, :], SP[:, :],
                            start=False, stop=False))
                if wv_next is not None:
                    ops.append(lambda: nc.gpsimd.tensor_add(
                        RW[:, :], Go[:, :], wv_next))
                return ops

            def emit_group(k0, wts, wts_next_chunk):
                """4 steps (k0..k0+3) for both chains, B staggered."""
                for j in range(4):
                    k = k0 + j
                    do_gate = k <= TR - 2
                    do_cost = k <= TR - 3
                    do_acc = do_cost and (j == 3)
                    opsl = []
                    for c, S in enumerate(chs):
                        wtile, base = wts[c]
                        if isinstance(base, int):
                            wv_cur = wtile[:, base + j * NL:base + (j + 1) * NL]
                        else:
                            wv_cur = wtile[:, bass.ds(base + j * NL, NL)]
                        if k + 1 <= TR - 1:
                            if j == 3 and wts_next_chunk is not None:
                                nwtile, nbase = wts_next_chunk[c]
                                wv_next = nwtile[:, nbase:nbase + NL]
                            elif isinstance(base, int):
                                nb = base + (j + 1) * NL
                                wv_next = wtile[:, nb:nb + NL]
                            else:
                                wv_next = wtile[:, bass.ds(base + (j + 1) * NL, NL)]
                        else:
                            wv_next = None
                        opsl.append(step_ops(S, j % 2, wv_cur, wv_next, j,
                                             do_gate, do_cost, do_acc))
                    sa, sb = opsl
                    off = 8   # stagger chain B ~half a step behind A
                    for i in range(max(len(sa), len(sb)) + off):
                        if i < len(sa):
                            sa[i]()
                        if 0 <= i - off < len(sb):
                            sb[i - off]()

            # chunks of 256 steps: 15 hw-loop bodies of 16 steps (240) +
            # 16 static tail steps (4 groups).
            for q in range(NQ):
                for c in range(NCH):
                    get_wtile(c, q)
                with tc.For_i(0, 15, 1) as iv:
                    for g in range(4):
                        wts = [(wtiles[(c, q)], iv * (16 * NL) + g * (4 * NL))
                               for c in range(NCH)]
                        emit_group(q * CH, wts, None)
                nxt = None
                if q + 1 < NQ:
                    nxt = [(get_wtile(c, q + 1), 0) for c in range(NCH)]
                for g in range(4):
                    k0 = q * CH + 240 + 4 * g
                    wts = [(wtiles[(c, q)], (240 + 4 * g) * NL)
                           for c in range(NCH)]
                    emit_group(k0, wts, nxt if g == 3 else None)

            # epilogue: after 2047 steps state parity lands in G[1]
            for c, S in enumerate(chs):
                Gl = S["G"][1]
                FSQ = sp.tile([16, NL], BF16, tag=f"FSQ{c}")
                nc.vector.tensor_mul(FSQ[:, :], Gl[0:16, :], Gl[0:16, :])
                nc.tensor.matmul(S["ACC"][:, :], wt["WFIN"][:, :], FSQ[:, :],
                                 start=False, stop=False)
                nc.tensor.matmul(S["ACC"][:, :], wt["WAC1D"][:, :],
                                 S["SG"][0:32, :], start=False, stop=False)
                nc.tensor.matmul(S["ACC"][:, :], wt["WAC1P"][:, :],
                                 S["SP"][0:32, :], start=False, stop=True)
                OUT = sp.tile([8, NL], F32, tag=f"OUT{c}")
                nc.scalar.activation(OUT[:, :], S["ACC"][:, :], AF.Copy,
                                     bias=float(init_c))
                nc.sync.dma_start(out_d[c].ap(), OUT[:, :])
    ctx.close()
    nc.compile()
    return nc


def _pack_w(w_core):
    """w_core [512, 2, 2047] f32 -> {(c,q): [32, 256*32] f32}."""
    out = {}
    T2 = NQ * CH
    for c in range(NCH):
        wc = w_core[c * LCH:(c + 1) * LCH].reshape(NG, NL, 2, TR)
        arr = np.zeros((32, T2, NL), np.float32)
        for g in range(NG):
            arr[g, :TR, :] = wc[g, :, 0, :].T         # x1 slot
            arr[8 + g, :TR, :] = wc[g, :, 1, :].T     # x2 slot
        for q in range(NQ):
            out[(c, q)] = np.ascontiguousarray(
                arr[:, q * CH:(q + 1) * CH, :]).reshape(
                    32, CH * NL).astype(ml_dtypes.bfloat16)
    return out


_PROG_CACHE = {}


def kernel(w, K, L, M, Mo):
    w = np.asarray(w, np.float32)
    K = np.asarray(K, np.float32)
    L = np.asarray(L, np.float32)
    M = np.asarray(M, np.float32)
    Mo = np.asarray(Mo, np.float32)
    B = w.shape[0]
    Wmats, Mo_f, init_c = _build_weights(K, L, M, Mo)

    key = (w.shape, K.tobytes(), L.tobytes(), M.tobytes(), Mo.tobytes())
    if key not in _PROG_CACHE:
        _PROG_CACHE[key] = _build_program(Mo_f, init_c)
    nc = _PROG_CACHE[key]

    xa, xb, gi = _init_consts()
    in_maps = []
    for core in range(N_CORES):
        m = {n: np.asarray(Wmats[n]) for n in Wmats}
        m["XIA"], m["XIB"], m["GI"] = xa, xb, gi
        wp = _pack_w(w[core * LPC:(core + 1) * LPC])
        for (c, q), arr in wp.items():
            m[f"w{c}_{q}"] = arr
        in_maps.append(m)

    kw = {}
    if os.environ.get("KERNEL_TRACE"):
        kw = dict(trace=True)
        if os.environ.get("KERNEL_TRACE_DIR"):
            kw["tmpdir"] = os.environ["KERNEL_TRACE_DIR"]
    res = bass_utils.run_bass_kernel_spmd(nc, in_maps,
                                          core_ids=list(range(N_CORES)), **kw)
    globals()["_LAST_RES"] = res
    out = np.empty(B, np.float32)
    for core in range(N_CORES):
        for c in range(NCH):
            o = res.results[core][f"out{c}"]       # [8, 32]
            lo = core * LPC + c * LCH
            out[lo:lo + LCH] = o.reshape(LCH)
    return out


# revision 52
# speedup vs baseline: 1.1825x; 1.0123x over previous
"""Trainium2 Bass kernel for nn_CSTR: B=4096-lane vmapped 2047-step rollout.

v3: data-parallel over 8 cores (512 lanes each), 2 independent 256-lane
chains per core, emission-staggered half a step apart so their
instruction streams fill each other's dependency gaps.

Per chain: slot-major layout, slot = 8 partitions x 32 free lanes,
state slots (x1, x2, xh1, xh2). Mega input tile X[104,32] bf16 =
[state@0; tanh@32; tanh^2@64; ones@96] (every engine-written sub-view
32-partition aligned). The full state update collapses to ONE bf16
matmul producing the O(H) increment Delta (u = K@xhat composed in, RK4
constants on the ones slot); the exact fp32 state G[32,32] is kept via
a vector add G' = Delta + (G + w). Gating: W_VD -> [Ls@rx + M;
0pad+(x-fp); Qc@x], products on DVE, W_PH -> phi (x4 dup), sigmoid on
ScalarE; the blend add uses a zero-padded delta*(x-fp) [32,32] so both
the fp32 state and bf16 mirror update with one aligned add each.
Stage cost accumulates into a persistent PSUM bank every 4 steps from
two [128,32] staging tiles (deltas, Qc products). Only O(H)-scaled
terms flow through bf16 matmuls; fp32 trajectory exactness is
preserved via the vector adds (2.7e-4 max rel err vs reference).
"""
import os
import sys
import numpy as np
import ml_dtypes
from contextlib import ExitStack

sys.path.insert(0, "/opt/trn_rl_repo")

import concourse.bacc as bacc
import concourse.bass as bass
import concourse.mybir as mybir
import concourse.tile as tile
from concourse import bass_utils

F32 = mybir.dt.float32
BF16 = mybir.dt.bfloat16
AF = mybir.ActivationFunctionType

H = np.float32(0.01)
LAM = np.float32(1.0)
B_TOT, N_CORES = 4096, 8
LPC = 512                 # lanes per core
NCH = 2                   # chains per core
LCH = 256                 # lanes per chain
NG, NL = 8, 32            # groups x free lanes per chain
TR = 2047                 # real steps
CH = 256                  # steps per w chunk
NQ = 8                    # chunks (last holds 255 real steps)

C1 = np.float32(1.0) - H
GC = np.float32(0.5) * H
EC = np.float32(H * H / 2)
A0 = np.float32(5e-5)
ALPHA = np.float32(H - np.float32(1e-6) / 3)


def _kron8(A):
    """lhsT for slot map A[out_slot, in_slot] -> [8*in, 8*out] bf16."""
    k = np.kron(np.ascontiguousarray(A.T), np.eye(NG, dtype=np.float32))
    return k.astype(ml_dtypes.bfloat16)


def _build_weights(K, L, M, Mo):
    K1, K2 = np.float32(K[0, 0]), np.float32(K[0, 1])
    Ls = ((L + L.T) * np.float32(0.5)).astype(np.float32)
    Mv = M[0].astype(np.float32)
    Qc = np.array([[1 + K1 * K1, K1 * K2], [K1 * K2, 1 + K2 * K2]], np.float32)
    # slot order (x1, x2, xh1, xh2) == rx order (x1, x2, fp1, fp2)

    # W_X inputs: state(4) T(4) Q(4) ones(1) = 13 slots -> Delta(4)
    uc = np.array([H, GC, H, GC], np.float32)
    Kv = np.array([0, 0, K1, K2], np.float32)
    A_S = (C1 - 1) * np.eye(4, dtype=np.float32) + np.outer(uc, Kv)
    A_T = np.zeros((4, 4), np.float32)
    A_T[0, 1] = ALPHA; A_T[1, 0] = -H; A_T[2, 3] = ALPHA; A_T[3, 2] = -H
    A_Q = np.zeros((4, 4), np.float32)
    A_Q[0, 1] = -A0; A_Q[2, 3] = -A0
    A_ONES = np.array([A0, -EC, A0, -EC], np.float32).reshape(4, 1)
    # 12-slot input basis (no Q): [state(4); T(4); ones(1); pad(3)]
    WX = np.concatenate(
        [A_S, A_T, A_ONES, np.zeros((4, 3), np.float32)], axis=1)  # [4, 12]

    # W_VD: 12 input slots -> [y(4); 0pad(2); DIF(2); y2(2)] = 10 out,
    # composed with the state update so it reads the PRE-update X tile:
    # VD = (Avd + AV4@A_WX)@X_old + AV4@w. The A0-scaled Q-terms of the
    # composition (~2.5e-6) are dropped so VD needn't wait for square.
    Avd = np.zeros((10, 12), np.float32)
    for s in range(4):
        for sp in range(4):
            Avd[s, sp] = Ls[s, sp]
        Avd[s, 8] = Mv[s]
    Avd[6, 0] = 1; Avd[6, 2] = -1          # DIF1 = x1 - xh1
    Avd[7, 1] = 1; Avd[7, 3] = -1          # DIF2 = x2 - xh2
    Avd[8, 0] = Qc[0, 0]; Avd[8, 1] = Qc[0, 1]
    Avd[9, 0] = Qc[1, 0]; Avd[9, 1] = Qc[1, 1]
    AV4 = Avd[:, 0:4]
    WVDX = Avd + AV4 @ WX                  # [10, 12]
    WVDW = AV4                             # [10, 4] applied to (w1,w2,0,0)

    APH = np.ones((4, 4), np.float32)      # phi, x4 dup
    AC4D = np.zeros((1, 16), np.float32)
    AC4P = np.zeros((1, 16), np.float32)
    for b in range(4):
        AC4D[0, 4 * b] = LAM
        AC4P[0, 4 * b] = 1; AC4P[0, 4 * b + 1] = 1
    AC1D = np.zeros((1, 4), np.float32); AC1D[0, 0] = LAM
    AC1P = np.zeros((1, 4), np.float32); AC1P[0, 0] = 1; AC1P[0, 1] = 1
    AFIN = np.array([[10.0, 10.0]], np.float32)

    W = {
        "WX": _kron8(WX),        # [96, 32]
        "WQ2": _kron8(A_Q),      # [32, 32]
        "WVD": _kron8(WVDX),     # [96, 80]
        "WVDW": _kron8(WVDW),    # [32, 80]
        "WPH": _kron8(APH),      # [32, 32]
        "WAC4D": _kron8(AC4D),   # [128, 8]
        "WAC4P": _kron8(AC4P),   # [128, 8]
        "WAC1D": _kron8(AC1D),   # [32, 8]
        "WAC1P": _kron8(AC1P),   # [32, 8]
        "WFIN": _kron8(AFIN),    # [16, 8]
    }
    init_c = float(1.0 + K1 * K1 + LAM)
    return W, float(Mo[0, 0]), init_c


WSHAPES = {"WX": (96, 32), "WQ2": (32, 32), "WVD": (96, 80),
           "WVDW": (32, 80), "WPH": (32, 32),
           "WAC4D": (128, 8), "WAC4P": (128, 8), "WAC1D": (32, 8),
           "WAC1P": (32, 8), "WFIN": (16, 8)}


def _init_consts():
    xa = np.zeros((96, NL), np.float32)
    for lo, v in ((0, 1.0), (8, 0.0), (16, 1.0), (24, 0.0)):   # state
        xa[lo:lo + 8] = v
    xa[64:72] = 1.0                                            # ones
    xb = np.zeros((96, NL), np.float32)
    xb[64:72] = 1.0
    gi = np.zeros((32, NL), np.float32)
    gi[0:8] = 1.0; gi[16:24] = 1.0
    return (xa.astype(ml_dtypes.bfloat16), xb.astype(ml_dtypes.bfloat16), gi)


def _build_program(Mo_f, init_c):
    nc = bacc.Bacc("TRN2", target_bir_lowering=False, debug=False)

    wd = {}
    for c in range(NCH):
        for q in range(NQ):
            wd[(c, q)] = nc.dram_tensor(
                f"w{c}_{q}", [32, CH * NL], BF16, kind="ExternalInput")
    wdram = {n: nc.dram_tensor(n, list(WSHAPES[n]), BF16, kind="ExternalInput")
             for n in WSHAPES}
    xa_d = nc.dram_tensor("XIA", [96, NL], BF16, kind="ExternalInput")
    xb_d = nc.dram_tensor("XIB", [96, NL], BF16, kind="ExternalInput")
    gi_d = nc.dram_tensor("GI", [32, NL], F32, kind="ExternalInput")
    out_d = [nc.dram_tensor(f"out{c}", [NG, NL], F32, kind="ExternalOutput")
             for c in range(NCH)]

    ctx = ExitStack()
    with tile.TileContext(nc) as tc:
        with tc.tile_pool(name="wq", bufs=2) as wpool, \
             tc.tile_pool(name="wt", bufs=1) as cpool, \
             tc.tile_pool(name="sb", bufs=3) as sp, \
             tc.tile_pool(name="ps", bufs=1, space="PSUM") as pp, \
             tc.tile_pool(name="pacc", bufs=1, space="PSUM") as pa:

            wt = {}
            for n in WSHAPES:
                wt[n] = cpool.tile(list(WSHAPES[n]), BF16, tag=n, name=n + "_t")
                nc.sync.dma_start(wt[n][:, :], wdram[n].ap())

            wtiles = {}

            def get_wtile(c, q):
                if (c, q) not in wtiles:
                    t = wpool.tile([32, CH * NL], BF16, tag=f"w{c}",
                                   name=f"w{c}_{q}_t")
                    nc.sync.dma_start(t[:, :], wd[(c, q)].ap())
                    wtiles[(c, q)] = t
                return wtiles[(c, q)]

            chs = []
            for c in range(NCH):
                S = {}
                S["X"] = [cpool.tile([96, NL], BF16, tag=f"XA{c}", name=f"XA{c}"),
                          cpool.tile([96, NL], BF16, tag=f"XB{c}", name=f"XB{c}")]
                S["G"] = [cpool.tile([32, NL], F32, tag=f"GA{c}", name=f"GA{c}"),
                          cpool.tile([32, NL], F32, tag=f"GB{c}", name=f"GB{c}")]
                S["RW"] = cpool.tile([32, NL], F32, tag=f"RW{c}", name=f"RW{c}")
                S["SG"] = cpool.tile([128, NL], BF16, tag=f"SG{c}", name=f"SG{c}")
                S["SP"] = cpool.tile([128, NL], BF16, tag=f"SP{c}", name=f"SP{c}")
                S["MOB"] = cpool.tile([32, 1], F32, tag=f"MOB{c}", name=f"MOB{c}")
                S["ACC"] = pa.tile([8, NL], F32, tag=f"ACC{c}", name=f"ACC{c}")
                S["tagD"] = f"Dp{c}"
                S["tagV"] = f"VDp{c}"
                S["tagP"] = f"PHp{c}"
                S["tagPP"] = f"PP{c}"
                S["tagM"] = f"M1{c}"
                S["tagQ"] = f"QT{c}"
                chs.append(S)

            Z = cpool.tile([32, NL], BF16, tag="Z", name="Z")
            nc.vector.memset(Z[:, :], 0.0)

            for c, S in enumerate(chs):
                nc.sync.dma_start(S["X"][0][:, :], xa_d.ap())
                nc.sync.dma_start(S["X"][1][:, :], xb_d.ap())
                nc.sync.dma_start(S["G"][0][:, :], gi_d.ap())
                nc.vector.memset(S["SG"][:, :], 0.0)
                nc.gpsimd.memset(S["SP"][:, :], 0.0)
                nc.vector.memset(S["MOB"][:, :], Mo_f)
                nc.tensor.matmul(S["ACC"][:, :], wt["WAC1D"][:, :], Z[:, :],
                                 start=True, stop=False)
                w0 = get_wtile(c, 0)
                nc.vector.tensor_add(S["RW"][:, :], S["G"][0][:, :],
                                     w0[:, 0:NL])
                if c == 1:
                    # one-time ~1.8us delay on chain B's first state add:
                    # forces the two chains into anti-phase so their ops
                    # fill each other's dependency gaps instead of
                    # contending in lockstep.
                    for _ in range(10):
                        nc.vector.tensor_add(S["RW"][:, :], S["RW"][:, :],
                                             Z[:, :])
                get_wtile(c, 1)

            def step_ops(S, p, wv_cur, wv_next, b, do_gate, do_cost, do_acc):
                """One step of one chain; p = input parity, b = SG block."""
                Xi, Xo = S["X"][p], S["X"][1 - p]
                Go = S["G"][1 - p]
                RW, SG, SP = S["RW"], S["SG"], S["SP"]
                Dp = pp.tile([32, NL], F32, tag=S["tagD"])
                ops = []
                ops.append(lambda: nc.scalar.activation(
                    Xi[32:64, :], Xi[0:32, :], AF.Tanh))
                QT = sp.tile([32, NL], BF16, tag=S["tagQ"])
                ops.append(lambda: nc.scalar.activation(
                    QT[:, :], Xi[32:64, :], AF.Square))
                ops.append(lambda: nc.tensor.matmul(
                    Dp[:, :], wt["WX"][:, :], Xi[0:96, :],
                    start=True, stop=False))
                if do_gate:
                    VDp = pp.tile([80, NL], F32, tag=S["tagV"])
                    ops.append(lambda: nc.tensor.matmul(
                        VDp[:, :], wt["WVD"][:, :], Xi[0:96, :],
                        start=True, stop=False))
                    ops.append(lambda: nc.tensor.matmul(
                        VDp[:, :], wt["WVDW"][:, :], wv_cur,
                        start=False, stop=True))
                ops.append(lambda: nc.tensor.matmul(
                    Dp[:, :], wt["WQ2"][:, :], QT[:, :],
                    start=False, stop=True))
                ops.append(lambda: nc.vector.tensor_add(
                    Xo[0:32, :], Dp[:, :], RW[:, :]))
                ops.append(lambda: nc.vector.tensor_add(
                    Go[:, :], Dp[:, :], RW[:, :]))
                if do_gate:
                    PHp = pp.tile([32, NL], F32, tag=S["tagP"])
                    PP = sp.tile([32, NL], BF16, tag=S["tagPP"])
                    M1 = sp.tile([32, NL], F32, tag=S["tagM"])
                    ops.append(lambda: nc.vector.tensor_mul(
                        PP[:, :], Xo[0:32, :], VDp[0:32, :]))
                    if do_cost:
                        ops.append(lambda: nc.vector.tensor_mul(
                            SP[32 * b:32 * b + 16, :],
                            Xo[0:16, :], VDp[64:80, :]))
                    ops.append(lambda: nc.tensor.matmul(
                        PHp[:, :], wt["WPH"][:, :], PP[:, :],
                        start=True, stop=True))
                    ops.append(lambda: nc.scalar.activation(
                        SG[32 * b:32 * b + 32, :], PHp[:, :], AF.Sigmoid,
                        bias=S["MOB"][:, :]))
                    ops.append(lambda: nc.vector.tensor_mul(
                        M1[:, :], SG[32 * b:32 * b + 32, :], VDp[32:64, :]))
                    ops.append(lambda: nc.vector.tensor_add(
                        Xo[0:32, :], Xo[0:32, :], M1[:, :]))
                    ops.append(lambda: nc.gpsimd.tensor_add(
                        Go[:, :], Go[:, :], M1[:, :]))
                    if do_acc:
                        ops.append(lambda: nc.tensor.matmul(
                            S["ACC"][:, :], wt["WAC4D"][:, :], SG[:, :],
                            start=False, stop=False))
                        ops.append(lambda: nc.tensor.matmul(
                            S["ACC"][:, :], wt["WAC4P"][:, :], SP[:, :],
                            start=False, stop=False))
                if wv_next is not None:
                    ops.append(lambda: nc.gpsimd.tensor_add(
                        RW[:, :], Go[:, :], wv_next))
                return ops

            def emit_group(k0, wts, wts_next_chunk):
                """4 steps (k0..k0+3) for both chains, B staggered."""
                for j in range(4):
                    k = k0 + j
                    do_gate = k <= TR - 2
                    do_cost = k <= TR - 3
                    do_acc = do_cost and (j == 3)
                    opsl = []
                    for c, S in enumerate(chs):
                        wtile, base = wts[c]
                        if isinstance(base, int):
                            wv_cur = wtile[:, base + j * NL:base + (j + 1) * NL]
                        else:
                            wv_cur = wtile[:, bass.ds(base + j * NL, NL)]
                        if k + 1 <= TR - 1:
                            if j == 3 and wts_next_chunk is not None:
                                nwtile, nbase = wts_next_chunk[c]
                                wv_next = nwtile[:, nbase:nbase + NL]
                            elif isinstance(base, int):
                                nb = base + (j + 1) * NL
                                wv_next = wtile[:, nb:nb + NL]
                            else:
                                wv_next = wtile[:, bass.ds(base + (j + 1) * NL, NL)]
                        else:
                            wv_next = None
                        opsl.append(step_ops(S, j % 2, wv_cur, wv_next, j,
                                             do_gate, do_cost, do_acc))
                    sa, sb = opsl
                    off = 8   # stagger chain B ~half a step behind A
                    for i in range(max(len(sa), len(sb)) + off):
                        if i < len(sa):
                            sa[i]()
                        if 0 <= i - off < len(sb):
                            sb[i - off]()

            # chunks of 256 steps: 15 hw-loop bodies of 16 steps (240) +
            # 16 static tail steps (4 groups).
            for q in range(NQ):
                for c in range(NCH):
                    get_wtile(c, q)
                with tc.For_i(0, 15, 1) as iv:
                    for g in range(4):
                        wts = [(wtiles[(c, q)], iv * (16 * NL) + g * (4 * NL))
                               for c in range(NCH)]
                        emit_group(q * CH, wts, None)
                nxt = None
                if q + 1 < NQ:
                    nxt = [(get_wtile(c, q + 1), 0) for c in range(NCH)]
                for g in range(4):
                    k0 = q * CH + 240 + 4 * g
                    wts = [(wtiles[(c, q)], (240 + 4 * g) * NL)
                           for c in range(NCH)]
                    emit_group(k0, wts, nxt if g == 3 else None)

            # epilogue: after 2047 steps state parity lands in G[1]
            for c, S in enumerate(chs):
                Gl = S["G"][1]
                FSQ = sp.tile([16, NL], BF16, tag=f"FSQ{c}")
                nc.vector.tensor_mul(FSQ[:, :], Gl[0:16, :], Gl[0:16, :])
                nc.tensor.matmul(S["ACC"][:, :], wt["WFIN"][:, :], FSQ[:, :],
                                 start=False, stop=False)
                nc.tensor.matmul(S["ACC"][:, :], wt["WAC1D"][:, :],
                                 S["SG"][0:32, :], start=False, stop=False)
                nc.tensor.matmul(S["ACC"][:, :], wt["WAC1P"][:, :],
                                 S["SP"][0:32, :], start=False, stop=True)
                OUT = sp.tile([8, NL], F32, tag=f"OUT{c}")
                nc.scalar.activation(OUT[:, :], S["ACC"][:, :], AF.Copy,
                                     bias=float(init_c))
                nc.sync.dma_start(out_d[c].ap(), OUT[:, :])
    ctx.close()
    nc.compile()
    return nc


def _pack_w(w_core):
    """w_core [512, 2, 2047] f32 -> {(c,q): [32, 256*32] f32}."""
    out = {}
    T2 = NQ * CH
    for c in range(NCH):
        wc = w_core[c * LCH:(c + 1) * LCH].reshape(NG, NL, 2, TR)
        arr = np.zeros((32, T2, NL), np.float32)
        for g in range(NG):
            arr[g, :TR, :] = wc[g, :, 0, :].T         # x1 slot
            arr[8 + g, :TR, :] = wc[g, :, 1, :].T     # x2 slot
        for q in range(NQ):
            out[(c, q)] = np.ascontiguousarray(
                arr[:, q * CH:(q + 1) * CH, :]).reshape(
                    32, CH * NL).astype(ml_dtypes.bfloat16)
    return out


_PROG_CACHE = {}


def kernel(w, K, L, M, Mo):
    w = np.asarray(w, np.float32)
    K = np.asarray(K, np.float32)
    L = np.asarray(L, np.float32)
    M = np.asarray(M, np.float32)
    Mo = np.asarray(Mo, np.float32)
    B = w.shape[0]
    Wmats, Mo_f, init_c = _build_weights(K, L, M, Mo)

    key = (w.shape, K.tobytes(), L.tobytes(), M.tobytes(), Mo.tobytes())
    if key not in _PROG_CACHE:
        _PROG_CACHE[key] = _build_program(Mo_f, init_c)
    nc = _PROG_CACHE[key]

    xa, xb, gi = _init_consts()
    in_maps = []
    for core in range(N_CORES):
        m = {n: np.asarray(Wmats[n]) for n in Wmats}
        m["XIA"], m["XIB"], m["GI"] = xa, xb, gi
        wp = _pack_w(w[core * LPC:(core + 1) * LPC])
        for (c, q), arr in wp.items():
            m[f"w{c}_{q}"] = arr
        in_maps.append(m)

    kw = {}
    if os.environ.get("KERNEL_TRACE"):
        kw = dict(trace=True)
        if os.environ.get("KERNEL_TRACE_DIR"):
            kw["tmpdir"] = os.environ["KERNEL_TRACE_DIR"]
    res = bass_utils.run_bass_kernel_spmd(nc, in_maps,
                                          core_ids=list(range(N_CORES)), **kw)
    globals()["_LAST_RES"] = res
    out = np.empty(B, np.float32)
    for core in range(N_CORES):
        for c in range(NCH):
            o = res.results[core][f"out{c}"]       # [8, 32]
            lo = core * LPC + c * LCH
            out[lo:lo + LCH] = o.reshape(LCH)
    return out


# revision 55
# speedup vs baseline: 1.2464x; 1.0541x over previous
"""Trainium2 Bass kernel for nn_CSTR: B=4096-lane vmapped 2047-step rollout.

v3: data-parallel over 8 cores (512 lanes each), 2 independent 256-lane
chains per core, emission-staggered half a step apart so their
instruction streams fill each other's dependency gaps.

Per chain: slot-major layout, slot = 8 partitions x 32 free lanes,
state slots (x1, x2, xh1, xh2). Mega input tile X[104,32] bf16 =
[state@0; tanh@32; tanh^2@64; ones@96] (every engine-written sub-view
32-partition aligned). The full state update collapses to ONE bf16
matmul producing the O(H) increment Delta (u = K@xhat composed in, RK4
constants on the ones slot); the exact fp32 state G[32,32] is kept via
a vector add G' = Delta + (G + w). Gating: W_VD -> [Ls@rx + M;
0pad+(x-fp); Qc@x], products on DVE, W_PH -> phi (x4 dup), sigmoid on
ScalarE; the blend add uses a zero-padded delta*(x-fp) [32,32] so both
the fp32 state and bf16 mirror update with one aligned add each.
Stage cost accumulates into a persistent PSUM bank every 4 steps from
two [128,32] staging tiles (deltas, Qc products). Only O(H)-scaled
terms flow through bf16 matmuls; fp32 trajectory exactness is
preserved via the vector adds (2.7e-4 max rel err vs reference).
"""
import os
import sys
import numpy as np
import ml_dtypes
from contextlib import ExitStack

sys.path.insert(0, "/opt/trn_rl_repo")

import concourse.bacc as bacc
import concourse.bass as bass
import concourse.mybir as mybir
import concourse.tile as tile
from concourse import bass_utils

F32 = mybir.dt.float32
BF16 = mybir.dt.bfloat16
AF = mybir.ActivationFunctionType

H = np.float32(0.01)
LAM = np.float32(1.0)
B_TOT, N_CORES = 4096, 8
LPC = 512                 # lanes per core
NCH = 2                   # chains per core
LCH = 256                 # lanes per chain
NG, NL = 8, 32            # groups x free lanes per chain
TR = 2047                 # real steps
CH = 256                  # steps per w chunk
NQ = 8                    # chunks (last holds 255 real steps)

C1 = np.float32(1.0) - H
GC = np.float32(0.5) * H
EC = np.float32(H * H / 2)
A0 = np.float32(5e-5)
ALPHA = np.float32(H - np.float32(1e-6) / 3)


def _kron8(A):
    """lhsT for slot map A[out_slot, in_slot] -> [8*in, 8*out] bf16."""
    k = np.kron(np.ascontiguousarray(A.T), np.eye(NG, dtype=np.float32))
    return k.astype(ml_dtypes.bfloat16)


def _build_weights(K, L, M, Mo):
    K1, K2 = np.float32(K[0, 0]), np.float32(K[0, 1])
    Ls = ((L + L.T) * np.float32(0.5)).astype(np.float32)
    Mv = M[0].astype(np.float32)
    Qc = np.array([[1 + K1 * K1, K1 * K2], [K1 * K2, 1 + K2 * K2]], np.float32)
    # slot order (x1, x2, xh1, xh2) == rx order (x1, x2, fp1, fp2)

    # W_X inputs: state(4) T(4) Q(4) ones(1) = 13 slots -> Delta(4)
    uc = np.array([H, GC, H, GC], np.float32)
    Kv = np.array([0, 0, K1, K2], np.float32)
    A_S = (C1 - 1) * np.eye(4, dtype=np.float32) + np.outer(uc, Kv)
    A_T = np.zeros((4, 4), np.float32)
    A_T[0, 1] = ALPHA; A_T[1, 0] = -H; A_T[2, 3] = ALPHA; A_T[3, 2] = -H
    A_Q = np.zeros((4, 4), np.float32)
    A_Q[0, 1] = -A0; A_Q[2, 3] = -A0
    A_ONES = np.array([A0, -EC, A0, -EC], np.float32).reshape(4, 1)
    # 12-slot input basis (no Q): [state(4); T(4); ones(1); pad(3)]
    WX = np.concatenate(
        [A_S, A_T, A_ONES, np.zeros((4, 3), np.float32)], axis=1)  # [4, 12]

    # W_VD: 12 input slots -> [y(4); 0pad(2); DIF(2); y2(2)] = 10 out,
    # composed with the state update so it reads the PRE-update X tile:
    # VD = (Avd + AV4@A_WX)@X_old + AV4@w. The A0-scaled Q-terms of the
    # composition (~2.5e-6) are dropped so VD needn't wait for square.
    Avd = np.zeros((10, 12), np.float32)
    for s in range(4):
        for sp in range(4):
            Avd[s, sp] = Ls[s, sp]
        Avd[s, 8] = Mv[s]
    Avd[6, 0] = 1; Avd[6, 2] = -1          # DIF1 = x1 - xh1
    Avd[7, 1] = 1; Avd[7, 3] = -1          # DIF2 = x2 - xh2
    Avd[8, 0] = Qc[0, 0]; Avd[8, 1] = Qc[0, 1]
    Avd[9, 0] = Qc[1, 0]; Avd[9, 1] = Qc[1, 1]
    AV4 = Avd[:, 0:4]
    WVDX = Avd + AV4 @ WX                  # [10, 12]
    WVDW = AV4                             # [10, 4] applied to (w1,w2,0,0)

    APH = np.ones((4, 4), np.float32)      # phi, x4 dup
    AC4D = np.zeros((1, 16), np.float32)
    AC4P = np.zeros((1, 16), np.float32)
    for b in range(4):
        AC4D[0, 4 * b] = LAM
        AC4P[0, 4 * b] = 1; AC4P[0, 4 * b + 1] = 1
    AC1D = np.zeros((1, 4), np.float32); AC1D[0, 0] = LAM
    AC1P = np.zeros((1, 4), np.float32); AC1P[0, 0] = 1; AC1P[0, 1] = 1
    AFIN = np.array([[10.0, 10.0]], np.float32)

    W = {
        "WX": _kron8(WX),        # [96, 32]
        "WQ2": _kron8(A_Q),      # [32, 32]
        "WVD": _kron8(WVDX),     # [96, 80]
        "WVDW": _kron8(WVDW),    # [32, 80]
        "WPH": _kron8(APH),      # [32, 32]
        "WAC4D": _kron8(AC4D),   # [128, 8]
        "WAC4P": _kron8(AC4P),   # [128, 8]
        "WAC1D": _kron8(AC1D),   # [32, 8]
        "WAC1P": _kron8(AC1P),   # [32, 8]
        "WFIN": _kron8(AFIN),    # [16, 8]
    }
    init_c = float(1.0 + K1 * K1 + LAM)
    return W, float(Mo[0, 0]), init_c


WSHAPES = {"WX": (96, 32), "WQ2": (32, 32), "WVD": (96, 80),
           "WVDW": (32, 80), "WPH": (32, 32),
           "WAC4D": (128, 8), "WAC4P": (128, 8), "WAC1D": (32, 8),
           "WAC1P": (32, 8), "WFIN": (16, 8)}


def _init_consts():
    xa = np.zeros((96, NL), np.float32)
    for lo, v in ((0, 1.0), (8, 0.0), (16, 1.0), (24, 0.0)):   # state
        xa[lo:lo + 8] = v
    xa[64:72] = 1.0                                            # ones
    xb = np.zeros((96, NL), np.float32)
    xb[64:72] = 1.0
    gi = np.zeros((32, NL), np.float32)
    gi[0:8] = 1.0; gi[16:24] = 1.0
    return (xa.astype(ml_dtypes.bfloat16), xb.astype(ml_dtypes.bfloat16), gi)


def _build_program(Mo_f, init_c):
    nc = bacc.Bacc("TRN2", target_bir_lowering=False, debug=False)

    wd = {}
    for c in range(NCH):
        for q in range(NQ):
            wd[(c, q)] = nc.dram_tensor(
                f"w{c}_{q}", [32, CH * NL], BF16, kind="ExternalInput")
    wdram = {n: nc.dram_tensor(n, list(WSHAPES[n]), BF16, kind="ExternalInput")
             for n in WSHAPES}
    xa_d = nc.dram_tensor("XIA", [96, NL], BF16, kind="ExternalInput")
    xb_d = nc.dram_tensor("XIB", [96, NL], BF16, kind="ExternalInput")
    gi_d = nc.dram_tensor("GI", [32, NL], F32, kind="ExternalInput")
    out_d = [nc.dram_tensor(f"out{c}", [NG, NL], F32, kind="ExternalOutput")
             for c in range(NCH)]

    ctx = ExitStack()
    with tile.TileContext(nc) as tc:
        with tc.tile_pool(name="wq", bufs=2) as wpool, \
             tc.tile_pool(name="wt", bufs=1) as cpool, \
             tc.tile_pool(name="sb", bufs=3) as sp, \
             tc.tile_pool(name="ps", bufs=1, space="PSUM") as pp, \
             tc.tile_pool(name="pacc", bufs=1, space="PSUM") as pa:

            wt = {}
            for n in WSHAPES:
                wt[n] = cpool.tile(list(WSHAPES[n]), BF16, tag=n, name=n + "_t")
                nc.sync.dma_start(wt[n][:, :], wdram[n].ap())

            wtiles = {}

            def get_wtile(c, q):
                if (c, q) not in wtiles:
                    t = wpool.tile([32, CH * NL], BF16, tag=f"w{c}",
                                   name=f"w{c}_{q}_t")
                    nc.sync.dma_start(t[:, :], wd[(c, q)].ap())
                    wtiles[(c, q)] = t
                return wtiles[(c, q)]

            chs = []
            for c in range(NCH):
                S = {}
                S["X"] = [cpool.tile([96, NL], BF16, tag=f"XA{c}", name=f"XA{c}"),
                          cpool.tile([96, NL], BF16, tag=f"XB{c}", name=f"XB{c}")]
                S["G"] = [cpool.tile([32, NL], F32, tag=f"GA{c}", name=f"GA{c}"),
                          cpool.tile([32, NL], F32, tag=f"GB{c}", name=f"GB{c}")]
                S["RW"] = cpool.tile([32, NL], F32, tag=f"RW{c}", name=f"RW{c}")
                S["SG"] = cpool.tile([128, NL], BF16, tag=f"SG{c}", name=f"SG{c}")
                S["SP"] = cpool.tile([128, NL], BF16, tag=f"SP{c}", name=f"SP{c}")
                S["MOB"] = cpool.tile([32, 1], F32, tag=f"MOB{c}", name=f"MOB{c}")
                S["ACC"] = pa.tile([8, NL], F32, tag=f"ACC{c}", name=f"ACC{c}")
                S["tagD"] = f"Dp{c}"
                S["tagV"] = f"VDp{c}"
                S["tagP"] = f"PHp{c}"
                S["tagPP"] = f"PP{c}"
                S["tagM"] = f"M1{c}"
                S["tagQ"] = f"QT{c}"
                chs.append(S)

            Z = cpool.tile([32, NL], BF16, tag="Z", name="Z")
            nc.vector.memset(Z[:, :], 0.0)

            for c, S in enumerate(chs):
                nc.sync.dma_start(S["X"][0][:, :], xa_d.ap())
                nc.sync.dma_start(S["X"][1][:, :], xb_d.ap())
                nc.sync.dma_start(S["G"][0][:, :], gi_d.ap())
                nc.vector.memset(S["SG"][:, :], 0.0)
                nc.gpsimd.memset(S["SP"][:, :], 0.0)
                nc.vector.memset(S["MOB"][:, :], Mo_f)
                nc.tensor.matmul(S["ACC"][:, :], wt["WAC1D"][:, :], Z[:, :],
                                 start=True, stop=False)
                w0 = get_wtile(c, 0)
                nc.vector.tensor_add(S["RW"][:, :], S["G"][0][:, :],
                                     w0[:, 0:NL])
                if c == 1:
                    # one-time ~1.8us delay on chain B's first state add:
                    # forces the two chains into anti-phase so their ops
                    # fill each other's dependency gaps instead of
                    # contending in lockstep.
                    for _ in range(10):
                        nc.vector.tensor_add(S["RW"][:, :], S["RW"][:, :],
                                             Z[:, :])
                get_wtile(c, 1)

            def step_ops(S, p, wv_cur, wv_next, b, do_gate, do_cost, do_acc):
                """One step of one chain; p = input parity, b = SG block."""
                Xi, Xo = S["X"][p], S["X"][1 - p]
                Go = S["G"][1 - p]
                RW, SG, SP = S["RW"], S["SG"], S["SP"]
                Dp = pp.tile([32, NL], F32, tag=S["tagD"])
                ops = []
                ops.append(lambda: nc.scalar.activation(
                    Xi[32:64, :], Xi[0:32, :], AF.Tanh))
                QT = sp.tile([32, NL], BF16, tag=S["tagQ"])
                ops.append(lambda: nc.scalar.activation(
                    QT[:, :], Xi[32:64, :], AF.Square))
                ops.append(lambda: nc.tensor.matmul(
                    Dp[:, :], wt["WX"][:, :], Xi[0:96, :],
                    start=True, stop=False))
                if do_gate:
                    VDp = pp.tile([80, NL], F32, tag=S["tagV"])
                    ops.append(lambda: nc.tensor.matmul(
                        VDp[:, :], wt["WVD"][:, :], Xi[0:96, :],
                        start=True, stop=False))
                    ops.append(lambda: nc.tensor.matmul(
                        VDp[:, :], wt["WVDW"][:, :], wv_cur,
                        start=False, stop=True))
                ops.append(lambda: nc.tensor.matmul(
                    Dp[:, :], wt["WQ2"][:, :], QT[:, :],
                    start=False, stop=True))
                ops.append(lambda: nc.vector.tensor_add(
                    Xo[0:32, :], Dp[:, :], RW[:, :]))
                if do_gate:
                    PHp = pp.tile([32, NL], F32, tag=S["tagP"])
                    PP = sp.tile([32, NL], BF16, tag=S["tagPP"])
                    M1 = sp.tile([32, NL], F32, tag=S["tagM"])
                    ops.append(lambda: nc.vector.tensor_mul(
                        PP[:, :], Xo[0:32, :], VDp[0:32, :]))
                    if do_cost:
                        ops.append(lambda: nc.vector.tensor_mul(
                            SP[32 * b:32 * b + 16, :],
                            Xo[0:16, :], VDp[64:80, :]))
                ops.append(lambda: nc.vector.tensor_add(
                    Go[:, :], Dp[:, :], RW[:, :]))
                if do_gate:
                    ops.append(lambda: nc.tensor.matmul(
                        PHp[:, :], wt["WPH"][:, :], PP[:, :],
                        start=True, stop=True))
                    ops.append(lambda: nc.scalar.activation(
                        SG[32 * b:32 * b + 32, :], PHp[:, :], AF.Sigmoid,
                        bias=S["MOB"][:, :]))
                    ops.append(lambda: nc.vector.tensor_mul(
                        M1[:, :], SG[32 * b:32 * b + 32, :], VDp[32:64, :]))
                    ops.append(lambda: nc.vector.tensor_add(
                        Xo[0:32, :], Xo[0:32, :], M1[:, :]))
                    ops.append(lambda: nc.gpsimd.tensor_add(
                        Go[:, :], Go[:, :], M1[:, :]))
                    if do_acc:
                        ops.append(lambda: nc.tensor.matmul(
                            S["ACC"][:, :], wt["WAC4D"][:, :], SG[:, :],
                            start=False, stop=False))
                        ops.append(lambda: nc.tensor.matmul(
                            S["ACC"][:, :], wt["WAC4P"][:, :], SP[:, :],
                            start=False, stop=False))
                if wv_next is not None:
                    ops.append(lambda: nc.gpsimd.tensor_add(
                        RW[:, :], Go[:, :], wv_next))
                return ops

            def emit_group(k0, wts, wts_next_chunk):
                """4 steps (k0..k0+3) for both chains, B staggered."""
                sa_all, sb_all = [], []
                for j in range(4):
                    k = k0 + j
                    do_gate = k <= TR - 2
                    do_cost = k <= TR - 3
                    do_acc = do_cost and (j == 3)
                    opsl = []
                    for c, S in enumerate(chs):
                        wtile, base = wts[c]
                        if isinstance(base, int):
                            wv_cur = wtile[:, base + j * NL:base + (j + 1) * NL]
                        else:
                            wv_cur = wtile[:, bass.ds(base + j * NL, NL)]
                        if k + 1 <= TR - 1:
                            if j == 3 and wts_next_chunk is not None:
                                nwtile, nbase = wts_next_chunk[c]
                                wv_next = nwtile[:, nbase:nbase + NL]
                            elif isinstance(base, int):
                                nb = base + (j + 1) * NL
                                wv_next = wtile[:, nb:nb + NL]
                            else:
                                wv_next = wtile[:, bass.ds(base + (j + 1) * NL, NL)]
                        else:
                            wv_next = None
                        opsl.append(step_ops(S, j % 2, wv_cur, wv_next, j,
                                             do_gate, do_cost, do_acc))
                    sa_all.extend(opsl[0])
                    sb_all.extend(opsl[1])
                # continuous stagger across the whole 4-step group
                off = 8
                for i in range(max(len(sa_all), len(sb_all)) + off):
                    if i < len(sa_all):
                        sa_all[i]()
                    if 0 <= i - off < len(sb_all):
                        sb_all[i - off]()

            # chunks of 256 steps: 15 hw-loop bodies of 16 steps (240) +
            # 16 static tail steps (4 groups).
            for q in range(NQ):
                for c in range(NCH):
                    get_wtile(c, q)
                with tc.For_i(0, 15, 1) as iv:
                    for g in range(4):
                        wts = [(wtiles[(c, q)], iv * (16 * NL) + g * (4 * NL))
                               for c in range(NCH)]
                        emit_group(q * CH, wts, None)
                nxt = None
                if q + 1 < NQ:
                    nxt = [(get_wtile(c, q + 1), 0) for c in range(NCH)]
                for g in range(4):
                    k0 = q * CH + 240 + 4 * g
                    wts = [(wtiles[(c, q)], (240 + 4 * g) * NL)
                           for c in range(NCH)]
                    emit_group(k0, wts, nxt if g == 3 else None)

            # epilogue: after 2047 steps state parity lands in G[1]
            for c, S in enumerate(chs):
                Gl = S["G"][1]
                FSQ = sp.tile([16, NL], BF16, tag=f"FSQ{c}")
                nc.vector.tensor_mul(FSQ[:, :], Gl[0:16, :], Gl[0:16, :])
                nc.tensor.matmul(S["ACC"][:, :], wt["WFIN"][:, :], FSQ[:, :],
                                 start=False, stop=False)
                nc.tensor.matmul(S["ACC"][:, :], wt["WAC1D"][:, :],
                                 S["SG"][0:32, :], start=False, stop=False)
                nc.tensor.matmul(S["ACC"][:, :], wt["WAC1P"][:, :],
                                 S["SP"][0:32, :], start=False, stop=True)
                OUT = sp.tile([8, NL], F32, tag=f"OUT{c}")
                nc.scalar.activation(OUT[:, :], S["ACC"][:, :], AF.Copy,
                                     bias=float(init_c))
                nc.sync.dma_start(out_d[c].ap(), OUT[:, :])
    ctx.close()
    nc.compile()
    return nc


def _pack_w(w_core):
    """w_core [512, 2, 2047] f32 -> {(c,q): [32, 256*32] f32}."""
    out = {}
    T2 = NQ * CH
    for c in range(NCH):
        wc = w_core[c * LCH:(c + 1) * LCH].reshape(NG, NL, 2, TR)
        arr = np.zeros((32, T2, NL), np.float32)
        for g in range(NG):
            arr[g, :TR, :] = wc[g, :, 0, :].T         # x1 slot
            arr[8 + g, :TR, :] = wc[g, :, 1, :].T     # x2 slot
        for q in range(NQ):
            out[(c, q)] = np.ascontiguousarray(
                arr[:, q * CH:(q + 1) * CH, :]).reshape(
                    32, CH * NL).astype(ml_dtypes.bfloat16)
    return out


_PROG_CACHE = {}


def kernel(w, K, L, M, Mo):
    w = np.asarray(w, np.float32)
    K = np.asarray(K, np.float32)
    L = np.asarray(L, np.float32)
    M = np.asarray(M, np.float32)
    Mo = np.asarray(Mo, np.float32)
    B = w.shape[0]
    Wmats, Mo_f, init_c = _build_weights(K, L, M, Mo)

    key = (w.shape, K.tobytes(), L.tobytes(), M.tobytes(), Mo.tobytes())
    if key not in _PROG_CACHE:
        _PROG_CACHE[key] = _build_program(Mo_f, init_c)
    nc = _PROG_CACHE[key]

    xa, xb, gi = _init_consts()
    in_maps = []
    for core in range(N_CORES):
        m = {n: np.asarray(Wmats[n]) for n in Wmats}
        m["XIA"], m["XIB"], m["GI"] = xa, xb, gi
        wp = _pack_w(w[core * LPC:(core + 1) * LPC])
        for (c, q), arr in wp.items():
            m[f"w{c}_{q}"] = arr
        in_maps.append(m)

    kw = {}
    if os.environ.get("KERNEL_TRACE"):
        kw = dict(trace=True)
        if os.environ.get("KERNEL_TRACE_DIR"):
            kw["tmpdir"] = os.environ["KERNEL_TRACE_DIR"]
    res = bass_utils.run_bass_kernel_spmd(nc, in_maps,
                                          core_ids=list(range(N_CORES)), **kw)
    globals()["_LAST_RES"] = res
    out = np.empty(B, np.float32)
    for core in range(N_CORES):
        for c in range(NCH):
            o = res.results[core][f"out{c}"]       # [8, 32]
            lo = core * LPC + c * LCH
            out[lo:lo + LCH] = o.reshape(LCH)
    return out
